# revision 17
# baseline (speedup 1.0000x reference)
"""ConvNeXtV2 block (B=32, C=256, T=4096, K=9, H=512) on 8 trn2 cores.

Data-parallel over batch: 4 samples per core, no collectives.

v6 design notes (v5 was 574us, DVE-bound at 80% by the STT dwconv):
- fp8(e4m3) everywhere on the matmul path, exploiting DoubleRow perf
  mode (2 fp8 contraction rows per PE cell):
  * dwconv: back on the PE as 4 DoubleRow diag-pair matmuls + 1 plain
    fp8 matmul per (cc, 512-block) -- 5 column streams instead of 9.
    The tap pair (k, k+1) needs rhs rows at element stride 1, which the
    ifmap AP rejects (stride-1 hard-hangs the PE), so x is DMA'd TWICE
    into one SBUF tile: copy A (padded x) at 0 and copy B (x shifted by
    one) at a 16-aligned offset; the pair stride is then 4112. The
    extra HBM read is free - DMA is nowhere near roofline.
  * pw1: one DoubleRow matmul contracts all of C=256 per (hc, blk).
  * pw2: two DoubleRow matmuls contract H=512 per (cc, blk).
  * LN stats: S and Q each via one DoubleRow ones-pair matmul per block
    (lhsT = [1|...15 zeros...|1] fp8 row pair at stride 16).
- y (dwconv out) kept as one [128, 2T] fp8 tile per sample; the LN
  "-mu*r" rank-1 fold is gone: norm computes y = y*r + nmr with two
  tensor_tensor ops against PSUM-broadcast rows (r and nmr).
- x input fp8, block output y*8 in fp8, residual added on host in f32
  (unchanged from v5; I/O is 67MB/call vs 268MB for f32 in/out).
- no gpsimd/SWDGE anywhere; all DMAs are HWDGE (sync).
Host pre-folds ln_w/ln_b into pw1, grn_beta and the fp8 x8 scale into
the pw2 bias/gamma; dwconv taps and pw1 weights are pre-quantized to
fp8 on the host (LN washes the ~4% dwconv error; the residual dilutes
everything by ~12x).
"""

from contextlib import ExitStack

import ml_dtypes
import numpy as np

import concourse.bass as bass
import concourse.mybir as mybir
import concourse.tile as tile
from concourse import bacc
from concourse.bass_utils import run_bass_kernel_spmd

B, C, T, K, H = 32, 256, 4096, 9, 512
NCORES = 8
BL = B // NCORES          # samples per core
P = 128
NCC = C // P              # 2 channel chunks
NHC = H // P              # 4 hidden chunks
NBLK = T // 512           # 8 column blocks of 512
HALF = K // 2             # 4
HT = T // 2               # 2048 columns per half-row
F32 = mybir.dt.float32
BF16 = mybir.dt.bfloat16
FP8 = mybir.dt.float8e4
I32 = mybir.dt.int32
BF = ml_dtypes.bfloat16
F8 = ml_dtypes.float8_e4m3
ALU = mybir.AluOpType
AF = mybir.ActivationFunctionType
PM = mybir.MatmulPerfMode

OSCALE = 8.0              # block output scaled by 8 before the fp8 write

_CACHE = {}
_REPEAT = 1    # timing-only knob: emit the whole pipeline N times in one NEFF
_PFX = [""]    # tile-name suffix per repeat (names must be unique)

# xbw layout: copy A (padded x, 4105 elems) at 0, copy B (=A shifted by
# one element) at XBOFF (16-aligned so the DoubleRow pair stride is legal)
XA = 4105
XBOFF = 4112
XW = XBOFF + 4104

# cpack layout
_NF32 = 2 + 4 + 4 + 2                          # dwb, b1f, gam8, b2c8
_BF0 = _NF32 * 2                               # bf16 elem offset (=24)
_NBF = 1024 + 1 + 128                          # w2t, ones_col, ones_row
_F80 = _NF32 * 4 + _NBF * 2                    # fp8 byte offset (=2354)
_DG = _F80                                     # diag pairs: 4 pairs x 2cc x 256
_D8 = _DG + 2048                               # tap-8 diags: 2cc x 128
_W1 = _D8 + 256                                # w1pk: 1024
_O16 = _W1 + 1024                              # ones16: 17
CPB = _O16 + 17
CPB += (-CPB) % 4


def _ap3(t, off, s1, n2, s2):
    """[128, 2, n2] AP over tile t at element offset off (pair stride s1,
    inner stride s2) -- the 3D form DoubleRow matmuls consume."""
    v = t[:, off:off + 1]
    c = v.copy()
    pstride = list(c.ap[0])
    c.ap[:] = [pstride, [s1, 2], [s2, n2]]
    return c


def _rsqrt(nc, pool, v, pdim, n, tag):
    """Newton rsqrt on DVE for a small [pdim, n] f32 tile (avoids the ACT
    sqrt table set; gelu set stays resident)."""
    vi = pool.tile([pdim, n], I32, tag=f"{tag}_i", name=f"{tag}_i")
    nc.vector.tensor_scalar(
        out=vi, in0=v.bitcast(I32), scalar1=1, scalar2=None,
        op0=ALU.logical_shift_right,
    )
    nc.vector.tensor_scalar(out=vi, in0=vi, scalar1=0x5F3759DF, scalar2=-1,
                            op0=ALU.subtract, op1=ALU.mult)
    r = pool.tile([pdim, n], F32, tag=f"{tag}_r", name=f"{tag}_r")
    nc.vector.tensor_copy(out=r, in_=vi.bitcast(F32))
    h = pool.tile([pdim, n], F32, tag=f"{tag}_h", name=f"{tag}_h")
    for _ in range(3):
        nc.vector.tensor_mul(out=h, in0=r, in1=r)
        nc.vector.tensor_mul(out=h, in0=h, in1=v)
        nc.vector.tensor_scalar(
            out=h, in0=h, scalar1=-0.5, scalar2=1.5, op0=ALU.mult, op1=ALU.add
        )
        nc.vector.tensor_mul(out=r, in0=r, in1=h)
    return r


def _build():
    nc = bacc.Bacc(
        "TRN2", target_bir_lowering=False, debug=False, num_devices=NCORES
    )
    x_d = nc.dram_tensor("x", [BL, C, T], FP8, kind="ExternalInput").ap()
    cpack_d = nc.dram_tensor("cpack", [P, CPB], mybir.dt.uint8,
                             kind="ExternalInput").ap()
    out_d = nc.dram_tensor("out", [BL, C, T], FP8, kind="ExternalOutput").ap()

    with tile.TileContext(nc) as tc:
        with ExitStack() as ctx:
            _emit(ctx, tc, nc, x_d, out_d, cpack_d)
    nc.compile()
    return nc


def _emit(ctx, tc, nc, x_d, out_d, cpack_d):
    const = ctx.enter_context(tc.tile_pool(name="const", bufs=1))
    xb_p = ctx.enter_context(tc.tile_pool(name="xb", bufs=4))
    y_p = ctx.enter_context(tc.tile_pool(name="y", bufs=2))
    ysq_p = ctx.enter_context(tc.tile_pool(name="ysq", bufs=1))
    ypk_p = ctx.enter_context(tc.tile_pool(name="ypk", bufs=2))
    tmp_p = ctx.enter_context(tc.tile_pool(name="tmp", bufs=2))
    hid_p = ctx.enter_context(tc.tile_pool(name="hid", bufs=2))
    sm_p = ctx.enter_context(tc.tile_pool(name="sm", bufs=2))
    row_p = ctx.enter_context(tc.tile_pool(name="row", bufs=1))
    w2s_p = ctx.enter_context(tc.tile_pool(name="w2s", bufs=2))
    ob_p = ctx.enter_context(tc.tile_pool(name="ob", bufs=3))

    dw_ps = ctx.enter_context(tc.tile_pool(name="dwps", bufs=1, space="PSUM"))
    st_ps = ctx.enter_context(tc.tile_pool(name="stps", bufs=1, space="PSUM"))
    mm_ps = ctx.enter_context(tc.tile_pool(name="mmps", bufs=4, space="PSUM"))
    rep_ps = ctx.enter_context(tc.tile_pool(name="repps", bufs=2, space="PSUM"))

    # ---- constants: ONE packed DMA, then bitcast slices ----
    cp = const.tile([P, CPB], mybir.dt.uint8)
    nc.sync.dma_start(out=cp, in_=cpack_d)
    cpf = cp.bitcast(F32)
    dwb_s = cpf[:, 0:2]
    b1f_s = cpf[:, 2:6]
    gam_s = cpf[:, 6:10]              # grn gamma, pre-scaled by OSCALE
    b2c_s = cpf[:, 10:12]             # pw2 bias (+W2@grn_beta), pre-scaled
    cpb = cp.bitcast(BF16)
    w2t_s = cpb[:, _BF0:_BF0 + 1024]
    ones_col = cpb[:, _BF0 + 1024:_BF0 + 1025]
    ones_row = cpb[0:1, _BF0 + 1025:_BF0 + 1025 + P]
    cp8 = cp.bitcast(FP8)

    xb = {}       # (s, cc) -> fp8 [P, XW] padded input (copies A and B)
    y8 = {}       # s -> bf16 [P, 2T]  (cc-major, raw dwconv out)
    ypks = {}     # s -> fp8 [P, 2T]  (normed, the pw1 DoubleRow operand)
    hid = {}      # s -> fp8 [P, 4T]  (hc-major)
    rows = {}     # s -> (r_row, nmr_row) bf16 [1, T]
    w2s = {}      # s -> scaled pw2 lhsT (fp8)

    def load(s):
        for cc in range(NCC):
            t = xb_p.tile([P, XW], FP8, tag="xb", name=f"xb_{s}_{cc}{_PFX[0]}")
            xb[(s, cc)] = t
            src = x_d[s, cc * P:(cc + 1) * P, :]
            nc.sync.dma_start(out=t[:, HALF:HALF + T], in_=src)
            nc.sync.dma_start(out=t[:, XBOFF + 3:XBOFF + 3 + T], in_=src)
            # halos: A = [x0 x0 x0 x0 | x | x_ x_ x_ x_ x_], B = A shifted 1
            nc.vector.tensor_copy(
                out=t[:, 0:HALF],
                in_=t[:, HALF:HALF + 1].to_broadcast((P, HALF)))
            nc.vector.tensor_copy(
                out=t[:, HALF + T:XA],
                in_=t[:, HALF + T - 1:HALF + T].to_broadcast((P, XA - HALF - T)))
            nc.vector.tensor_copy(
                out=t[:, XBOFF:XBOFF + 3],
                in_=t[:, XBOFF + 3:XBOFF + 4].to_broadcast((P, 3)))
            nc.vector.tensor_copy(
                out=t[:, XBOFF + 3 + T:XW],
                in_=t[:, XBOFF + 2 + T:XBOFF + 3 + T].to_broadcast(
                    (P, XW - XBOFF - 3 - T)))

    def ln_half(s, hf, sqt, r_row, nmr_row):
        # LN math for one T-half on compact [16,128] tiles; emitted as soon
        # as that half's stats are drained so the rep matmuls never stall.
        HL = T // 2
        s_c = sm_p.tile([16, P], BF16, tag=f"s_c{hf}", name=f"s_c_{s}_{hf}{_PFX[0]}")
        q_c = sm_p.tile([16, P], BF16, tag=f"q_c{hf}", name=f"q_c_{s}_{hf}{_PFX[0]}")
        nc.sync.dma_start(out=s_c, in_=sqt[0:1, hf * HL:(hf + 1) * HL])
        nc.sync.dma_start(out=q_c, in_=sqt[32:33, hf * HL:(hf + 1) * HL])
        mu = sm_p.tile([16, P], F32, tag=f"mu{hf}")
        nc.vector.tensor_scalar(out=mu, in0=s_c, scalar1=1.0 / C, scalar2=None,
                                op0=ALU.mult)
        var = sm_p.tile([16, P], F32, tag=f"var{hf}")
        nc.vector.tensor_mul(out=var, in0=mu, in1=mu)
        nc.vector.scalar_tensor_tensor(
            out=var, in0=q_c, scalar=1.0 / C, in1=var,
            op0=ALU.mult, op1=ALU.subtract)
        nc.vector.tensor_scalar(out=var, in0=var, scalar1=1e-5, scalar2=None,
                                op0=ALU.add)
        r = _rsqrt(nc, sm_p, var, 16, P, f"rs{hf}")
        nmr = sm_p.tile([16, P], F32, tag=f"nmr{hf}")
        nc.vector.scalar_tensor_tensor(out=nmr, in0=mu, scalar=-1.0, in1=r,
                                       op0=ALU.mult, op1=ALU.mult)
        r_bf = sm_p.tile([16, P], BF16, tag=f"r_bf{hf}")
        nc.vector.tensor_copy(out=r_bf, in_=r)
        nmr_bf = sm_p.tile([16, P], BF16, tag=f"nmr_bf{hf}")
        nc.vector.tensor_copy(out=nmr_bf, in_=nmr)
        nc.sync.dma_start(out=r_row[:, hf * HL:(hf + 1) * HL], in_=r_bf)
        nc.sync.dma_start(out=nmr_row[:, hf * HL:(hf + 1) * HL], in_=nmr_bf)

    def dw_stats(s):
        yt = y_p.tile([P, 2 * T], BF16, tag="y", name=f"y_{s}{_PFX[0]}")
        y8[s] = yt
        ypk = ypk_p.tile([P, 2 * T], FP8, tag="ypk", name=f"ypk_{s}{_PFX[0]}")
        ypks[s] = ypk
        ysq = ysq_p.tile([P, 2 * T], BF16, tag="ysq", name=f"ysq_{s}{_PFX[0]}")
        sqt = row_p.tile([33, T], BF16, tag="sqt", name=f"sqt_{s}{_PFX[0]}")
        r_row = row_p.tile([1, T], BF16, tag="r_row", name=f"r_row_{s}{_PFX[0]}")
        nmr_row = row_p.tile([1, T], BF16, tag="nmr_row",
                             name=f"nmr_row_{s}{_PFX[0]}")
        rows[s] = (r_row, nmr_row)
        for hh in range(2):
            for sb in range(NBLK // 2):
                blk = hh * (NBLK // 2) + sb
                lo = blk * 512
                for cc in range(NCC):
                    xt = xb[(s, cc)]
                    ps = dw_ps.tile([P, 512], F32, tag="dwps")
                    for p_ in range(4):
                        nc.tensor.matmul(
                            ps,
                            lhsT=_ap3(cp8, _DG + (p_ * NCC + cc) * 256, 128, 128, 1),
                            rhs=_ap3(xt, lo + 2 * p_, XBOFF, 512, 1),
                            start=(p_ == 0), stop=False, perf_mode=PM.DoubleRow)
                    nc.tensor.matmul(
                        ps, lhsT=cp8[:, _D8 + cc * P:_D8 + (cc + 1) * P],
                        rhs=xt[:, lo + 8:lo + 8 + 512],
                        start=False, stop=True)
                    # drain psum + dw bias -> y bf16; split across ACT and
                    # DVE (psum reads are column-rate-bound on both)
                    if cc == 0:
                        nc.scalar.activation(
                            out=yt[:, cc * T + lo:cc * T + lo + 512], in_=ps,
                            func=AF.Identity, bias=dwb_s[:, cc:cc + 1],
                            scale=1.0)
                    else:
                        nc.vector.tensor_scalar(
                            out=yt[:, cc * T + lo:cc * T + lo + 512], in0=ps,
                            scalar1=dwb_s[:, cc:cc + 1], scalar2=None,
                            op0=ALU.add)
                for cc in range(NCC):
                    nc.vector.tensor_mul(
                        out=ysq[:, cc * T + lo:cc * T + lo + 512],
                        in0=yt[:, cc * T + lo:cc * T + lo + 512],
                        in1=yt[:, cc * T + lo:cc * T + lo + 512])
                # S and Q chains in different PE column groups -> they run
                # concurrently (DoubleRow rejects 1-partition dst, so plain
                # fp8 matmuls per cc chunk; lhsT = ones16 col 0)
                st2 = st_ps.tile([64, 512], F32, tag="stps",
                                 name=f"st2_{s}_{blk}{_PFX[0]}")
                for cc in range(NCC):
                    nc.tensor.matmul(st2[0:1, :], lhsT=ones_col,
                                     rhs=yt[:, cc * T + lo:cc * T + lo + 512],
                                     start=(cc == 0), stop=(cc == NCC - 1),
                                     tile_position=(0, 0),
                                     skip_group_check=True)
                    nc.tensor.matmul(st2[32:33, :], lhsT=ones_col,
                                     rhs=ysq[:, cc * T + lo:cc * T + lo + 512],
                                     start=(cc == 0), stop=(cc == NCC - 1),
                                     tile_position=(0, 32),
                                     skip_group_check=True)
                # one drain covers S (row 0) and Q (row 32); psum-op cost
                # is column-rate-bound so 33 partitions ride along free
                nc.vector.tensor_copy(out=sqt[:, lo:lo + 512],
                                      in_=st2[0:33, :])
            ln_half(s, hh, sqt, r_row, nmr_row)

    def norm(s):
        # ypk = y*r + nmr (fp8, the DoubleRow pw1 operand); broadcasts via
        # PSUM-resident rep matmuls
        r_row, nmr_row = rows[s]
        yt = y8[s]
        ypk = ypks[s]
        for blk in range(NBLK):
            lo = blk * 512
            r_ps = rep_ps.tile([P, 512], F32, tag="repps")
            nc.tensor.matmul(r_ps, lhsT=ones_row, rhs=r_row[:, lo:lo + 512],
                             start=True, stop=True)
            n_ps = rep_ps.tile([P, 512], F32, tag="repps")
            nc.tensor.matmul(n_ps, lhsT=ones_row, rhs=nmr_row[:, lo:lo + 512],
                             start=True, stop=True)
            for cc in range(NCC):
                ysl = yt[:, cc * T + lo:cc * T + lo + 512]
                tm = tmp_p.tile([P, 512], BF16, tag="tmp")
                nc.vector.tensor_mul(out=tm, in0=ysl, in1=r_ps)
                nc.vector.tensor_add(out=ypk[:, cc * T + lo:cc * T + lo + 512],
                                     in0=tm, in1=n_ps)

    gx2s = {}

    def pw1(s):
        ht = hid_p.tile([P, 4 * T], FP8, tag="hid", name=f"hid_{s}{_PFX[0]}")
        hid[s] = ht
        yt = y8[s]
        ypk = ypks[s]
        for hc in range(NHC):
            for blk in range(NBLK):
                lo = blk * 512
                ps = mm_ps.tile([P, 512], F32, tag="mmps")
                nc.tensor.matmul(
                    ps, lhsT=_ap3(cp8, _W1 + hc * P, 512, 128, 1),
                    rhs=_ap3(ypk, lo, T, 512, 1),
                    start=True, stop=True, perf_mode=PM.DoubleRow)
                nc.scalar.activation(
                    out=ht[:, hc * T + lo:hc * T + lo + 512],
                    in_=ps, func=AF.Gelu, bias=b1f_s[:, hc:hc + 1], scale=1.0)
        # GRN square+accum on ACT; y8(s) is dead after pw1 -> scratch
        gx2 = sm_p.tile([P, NHC], F32, tag="gx2", name=f"gx2_{s}{_PFX[0]}")
        gx2s[s] = gx2
        for hc in range(NHC):
            if s == BL - 1 and hc >= 2:
                sq = yt[:, 0:T].bitcast(BF16)
                nc.vector.tensor_mul(out=sq, in0=ht[:, hc * T:(hc + 1) * T],
                                     in1=ht[:, hc * T:(hc + 1) * T])
                nc.vector.tensor_reduce(out=gx2[:, hc:hc + 1], in_=sq,
                                        axis=mybir.AxisListType.X, op=ALU.add)
            else:
                nc.scalar.activation(out=yt[:, 0:T],
                                     in_=ht[:, hc * T:(hc + 1) * T],
                                     func=AF.Square,
                                     accum_out=gx2[:, hc:hc + 1])

    def grn(s):
        gx2 = gx2s[s]
        gx2f = sm_p.tile([P, NHC], F32, tag="gx2f")
        nc.vector.tensor_scalar(out=gx2f, in0=gx2, scalar1=1e-30, scalar2=None,
                                op0=ALU.add)
        rg = _rsqrt(nc, sm_p, gx2f, P, NHC, "rg")
        gx = sm_p.tile([P, NHC], F32, tag="gx")
        nc.vector.tensor_mul(out=gx, in0=gx2f, in1=rg)      # gx = sqrt(gx2)
        gx_bf = sm_p.tile([P, NHC], BF16, tag="gx_bf")
        nc.vector.tensor_copy(out=gx_bf, in_=gx)
        # mean over all H=512 channels: ones-matmul -> [1,4] -> reduce
        gt_ps = st_ps.tile([1, NHC], F32, tag="stps", name=f"gt_{s}{_PFX[0]}")
        nc.tensor.matmul(gt_ps, lhsT=ones_col, rhs=gx_bf,
                         start=True, stop=True)
        g_row = sm_p.tile([1, NHC], F32, tag="g_row")
        nc.vector.tensor_copy(out=g_row, in_=gt_ps)
        tot = sm_p.tile([1, 1], F32, tag="tot")
        nc.vector.tensor_reduce(out=tot, in_=g_row, axis=mybir.AxisListType.X,
                                op=ALU.add)
        nc.vector.tensor_scalar(out=tot, in0=tot, scalar1=1.0 / H,
                                scalar2=1e-6, op0=ALU.mult, op1=ALU.add)
        rm_row = sm_p.tile([1, 1], F32, tag="rm_row")
        nc.vector.reciprocal(out=rm_row, in_=tot)
        rm_bf = sm_p.tile([1, 1], BF16, tag="rm_bf")
        nc.vector.tensor_copy(out=rm_bf, in_=rm_row)
        rm_ps = st_ps.tile([P, 1], F32, tag="stps", name=f"rm_{s}{_PFX[0]}")
        nc.tensor.matmul(rm_ps, lhsT=ones_row, rhs=rm_bf,
                         start=True, stop=True)
        rm = sm_p.tile([P, 1], F32, tag="rm")
        nc.vector.tensor_copy(out=rm, in_=rm_ps)
        # a = OSCALE * (gamma*nx + 1); gamma arrives pre-scaled by OSCALE
        a = sm_p.tile([P, NHC], F32, tag="a")
        nc.vector.tensor_scalar(out=a, in0=gx, scalar1=rm, scalar2=None,
                                op0=ALU.mult)
        nc.vector.scalar_tensor_tensor(out=a, in0=a, scalar=1.0, in1=gam_s,
                                       op0=ALU.bypass, op1=ALU.mult)
        nc.vector.tensor_scalar(out=a, in0=a, scalar1=OSCALE, scalar2=None,
                                op0=ALU.add)
        w2s[s] = w2s_p.tile([P, NHC * C], FP8, tag="w2s", name=f"w2s_{s}{_PFX[0]}")
        for hc in range(NHC):
            nc.vector.tensor_scalar(
                out=w2s[s][:, hc * C:(hc + 1) * C],
                in0=w2t_s[:, hc * C:(hc + 1) * C],
                scalar1=a[:, hc:hc + 1], scalar2=None, op0=ALU.mult)

    def pw2(s, last=False):
        ht = hid[s]
        for cc in range(NCC):
            for ob_i in range(2):          # two [P, 2048] fp8 out tiles per cc
                ob = ob_p.tile([P, HT], FP8, tag="ob")
                pss = [mm_ps.tile([P, 512], F32, tag="mmps",
                                  name=f"pw2ps_{s}_{cc}_{ob_i}_{sub}{_PFX[0]}")
                       for sub in range(4)]
                # j outer: each DoubleRow weight is loaded once and reused
                # across the four 512-blocks
                for j in range(2):         # hc pairs (0,1) and (2,3)
                    for sub in range(4):
                        lo = (ob_i * 4 + sub) * 512
                        nc.tensor.matmul(
                            pss[sub],
                            lhsT=_ap3(w2s[s], (2 * j) * C + cc * P, C, 128, 1),
                            rhs=_ap3(ht, (2 * j) * T + lo, T, 512, 1),
                            start=(j == 0), stop=(j == 1),
                            perf_mode=PM.DoubleRow)
                for sub in range(4):
                    # drain: fp8 out = psum + bias2 (scaled by OSCALE); on the
                    # final sample nothing overlaps, so split ACT/DVE
                    if last and sub % 2 == 1:
                        nc.vector.tensor_scalar(
                            out=ob[:, sub * 512:(sub + 1) * 512], in0=pss[sub],
                            scalar1=b2c_s[:, cc:cc + 1], scalar2=None,
                            op0=ALU.add)
                    else:
                        nc.scalar.activation(
                            out=ob[:, sub * 512:(sub + 1) * 512], in_=pss[sub],
                            func=AF.Identity, bias=b2c_s[:, cc:cc + 1],
                            scale=1.0)
                nc.sync.dma_start(
                    out=out_d[s, cc * P:(cc + 1) * P,
                              ob_i * HT:(ob_i + 1) * HT],
                    in_=ob)

    # deferred-GRN pipeline: iter s runs dw+stats(s) / grn+pw2(s-1) /
    # norm+pw1(s); gelu+square ACT tails of pw1(s) overlap dw(s+1), giving
    # the GRN chain a full iteration of slack before pw2(s) needs w2s.
    for rp in range(_REPEAT):
        _PFX[0] = f"_rp{rp}" if _REPEAT > 1 else ""
        load(0)
        for s in range(BL):
            if s + 1 < BL:
                load(s + 1)
            dw_stats(s)
            if s >= 1:
                grn(s - 1)
                pw2(s - 1)
            norm(s)
            pw1(s)
        grn(BL - 1)
        pw2(BL - 1, last=True)


def _prep_inputs(inputs):
    x = np.asarray(inputs["x"], np.float32)
    dw_w = np.asarray(inputs["dw_w"], np.float32)      # (C,1,K)
    dw_b = np.asarray(inputs["dw_b"], np.float32)
    ln_w = np.asarray(inputs["ln_w"], np.float32)
    ln_b = np.asarray(inputs["ln_b"], np.float32)
    pw1_w = np.asarray(inputs["pw1_w"], np.float32)    # (H,C)
    pw1_b = np.asarray(inputs["pw1_b"], np.float32)
    gg = np.asarray(inputs["grn_gamma"], np.float32)
    gb = np.asarray(inputs["grn_beta"], np.float32)
    pw2_w = np.asarray(inputs["pw2_w"], np.float32)    # (C,H)
    pw2_b = np.asarray(inputs["pw2_b"], np.float32)

    dwb = dw_b.reshape(NCC, P).T.copy()
    b1f = (pw1_b + pw1_w @ ln_b).reshape(NHC, P).T.copy()
    gam = (gg * OSCALE).reshape(NHC, P).T.copy()
    b2c = ((pw2_b + pw2_w @ gb) * OSCALE).reshape(NCC, P).T.copy()

    w2t = np.zeros((P, NHC * C), BF)
    for hc in range(NHC):
        w2t[:, hc * C:(hc + 1) * C] = \
            pw2_w[:, hc * P:(hc + 1) * P].T.astype(BF)
    onescol = np.ones((P, 1), BF)
    onesrow_blk = np.zeros((P, P), BF)
    onesrow_blk[0, :] = 1.0

    # fp8 block: dwconv diag pairs, tap-8 diags, w1pk, ones16
    dgpk = np.zeros((P, 2048), F8)
    for p_ in range(4):
        for cc in range(NCC):
            base = (p_ * NCC + cc) * 256
            for j in range(2):
                k = 2 * p_ + j
                dg = np.zeros((P, P), np.float32)
                np.fill_diagonal(dg, dw_w[cc * P:(cc + 1) * P, 0, k])
                dgpk[:, base + j * P:base + (j + 1) * P] = dg.astype(F8)
    d8 = np.zeros((P, 2 * P), F8)
    for cc in range(NCC):
        dg = np.zeros((P, P), np.float32)
        np.fill_diagonal(dg, dw_w[cc * P:(cc + 1) * P, 0, 8])
        d8[:, cc * P:(cc + 1) * P] = dg.astype(F8)
    w1f = pw1_w * ln_w[None, :]                        # (H,C)
    w1pk = np.zeros((P, 1024), F8)
    for cc in range(NCC):
        for hc in range(NHC):
            w1pk[:, cc * 512 + hc * P:cc * 512 + (hc + 1) * P] = \
                w1f[hc * P:(hc + 1) * P, cc * P:(cc + 1) * P].T.astype(F8)
    ones16 = np.zeros((P, 17), F8)
    ones16[:, 0] = 1.0
    ones16[:, 16] = 1.0

    cpack = np.concatenate([
        dwb.view(np.uint8), b1f.view(np.uint8), gam.view(np.uint8),
        b2c.view(np.uint8),
        w2t.view(np.uint8), onescol.view(np.uint8), onesrow_blk.view(np.uint8),
        dgpk.view(np.uint8), d8.view(np.uint8), w1pk.view(np.uint8),
        ones16.view(np.uint8)], axis=1)
    pad = CPB - cpack.shape[1]
    if pad:
        cpack = np.concatenate([cpack, np.zeros((P, pad), np.uint8)], axis=1)
    assert cpack.shape == (P, CPB), cpack.shape
    x8 = x.astype(F8)
    common = {"cpack": np.ascontiguousarray(cpack)}
    in_maps = []
    for i in range(NCORES):
        m = dict(common)
        m["x"] = x8[i * BL:(i + 1) * BL]
        in_maps.append(m)
    return in_maps, x


def kernel(**inputs):
    if "nc" not in _CACHE:
        _CACHE["nc"] = _build()
    nc = _CACHE["nc"]
    in_maps, x = _prep_inputs(inputs)
    res = run_bass_kernel_spmd(nc, in_maps, core_ids=list(range(NCORES)),
                               **_CACHE.get("run_kwargs", {}))
    _CACHE["last_result"] = res
    y8 = np.concatenate([np.asarray(res.results[i]["out"])
                         for i in range(NCORES)], axis=0)
    return x + y8.astype(np.float32) * (1.0 / OSCALE)


# revision 18
# speedup vs baseline: 1.0675x; 1.0675x over previous
"""ConvNeXtV2 block (B=32, C=256, T=4096, K=9, H=512) on 8 trn2 cores.

Data-parallel over batch: 4 samples per core, no collectives.

v6 design notes (v5 was 574us, DVE-bound at 80% by the STT dwconv):
- fp8(e4m3) everywhere on the matmul path, exploiting DoubleRow perf
  mode (2 fp8 contraction rows per PE cell):
  * dwconv: back on the PE as 4 DoubleRow diag-pair matmuls + 1 plain
    fp8 matmul per (cc, 512-block) -- 5 column streams instead of 9.
    The tap pair (k, k+1) needs rhs rows at element stride 1, which the
    ifmap AP rejects (stride-1 hard-hangs the PE), so x is DMA'd TWICE
    into one SBUF tile: copy A (padded x) at 0 and copy B (x shifted by
    one) at a 16-aligned offset; the pair stride is then 4112. The
    extra HBM read is free - DMA is nowhere near roofline.
  * pw1: one DoubleRow matmul contracts all of C=256 per (hc, blk).
  * pw2: two DoubleRow matmuls contract H=512 per (cc, blk).
  * LN stats: S and Q each via one DoubleRow ones-pair matmul per block
    (lhsT = [1|...15 zeros...|1] fp8 row pair at stride 16).
- y (dwconv out) kept as one [128, 2T] fp8 tile per sample; the LN
  "-mu*r" rank-1 fold is gone: norm computes y = y*r + nmr with two
  tensor_tensor ops against PSUM-broadcast rows (r and nmr).
- x input fp8, block output y*8 in fp8, residual added on host in f32
  (unchanged from v5; I/O is 67MB/call vs 268MB for f32 in/out).
- no gpsimd/SWDGE anywhere; all DMAs are HWDGE (sync).
Host pre-folds ln_w/ln_b into pw1, grn_beta and the fp8 x8 scale into
the pw2 bias/gamma; dwconv taps and pw1 weights are pre-quantized to
fp8 on the host (LN washes the ~4% dwconv error; the residual dilutes
everything by ~12x).
"""

from contextlib import ExitStack

import ml_dtypes
import numpy as np

import concourse.bass as bass
import concourse.mybir as mybir
import concourse.tile as tile
from concourse import bacc
from concourse.bass_utils import run_bass_kernel_spmd

B, C, T, K, H = 32, 256, 4096, 9, 512
NCORES = 8
BL = B // NCORES          # samples per core
P = 128
NCC = C // P              # 2 channel chunks
NHC = H // P              # 4 hidden chunks
NBLK = T // 512           # 8 column blocks of 512
HALF = K // 2             # 4
HT = T // 2               # 2048 columns per half-row
F32 = mybir.dt.float32
BF16 = mybir.dt.bfloat16
FP8 = mybir.dt.float8e4
I32 = mybir.dt.int32
BF = ml_dtypes.bfloat16
F8 = ml_dtypes.float8_e4m3
ALU = mybir.AluOpType
AF = mybir.ActivationFunctionType
PM = mybir.MatmulPerfMode

OSCALE = 8.0              # block output scaled by 8 before the fp8 write

_CACHE = {}
_REPEAT = 1    # timing-only knob: emit the whole pipeline N times in one NEFF
_PFX = [""]    # tile-name suffix per repeat (names must be unique)

# xbw layout: copy A (padded x, 4105 elems) at 0, copy B (=A shifted by
# one element) at XBOFF (16-aligned so the DoubleRow pair stride is legal)
XA = 4105
XBOFF = 4112
XW = XBOFF + 4104

# cpack layout
_NF32 = 2 + 4 + 4 + 2                          # dwb, b1f, gam8, b2c8
_BF0 = _NF32 * 2                               # bf16 elem offset (=24)
_NBF = 1024 + 1 + 128                          # w2t, ones_col, ones_row
_F80 = _NF32 * 4 + _NBF * 2                    # fp8 byte offset (=2354)
_DG = _F80                                     # diag pairs: 4 pairs x 2cc x 256
_D8 = _DG + 2048                               # tap-8 diags: 2cc x 128
_W1 = _D8 + 256                                # w1pk: 1024
_O16 = _W1 + 1024                              # ones16: 17
CPB = _O16 + 17
CPB += (-CPB) % 4


def _ap3(t, off, s1, n2, s2):
    """[128, 2, n2] AP over tile t at element offset off (pair stride s1,
    inner stride s2) -- the 3D form DoubleRow matmuls consume."""
    v = t[:, off:off + 1]
    c = v.copy()
    pstride = list(c.ap[0])
    c.ap[:] = [pstride, [s1, 2], [s2, n2]]
    return c


def _rsqrt(nc, pool, v, pdim, n, tag):
    """Newton rsqrt on DVE for a small [pdim, n] f32 tile (avoids the ACT
    sqrt table set; gelu set stays resident)."""
    vi = pool.tile([pdim, n], I32, tag=f"{tag}_i", name=f"{tag}_i")
    nc.vector.tensor_scalar(
        out=vi, in0=v.bitcast(I32), scalar1=1, scalar2=None,
        op0=ALU.logical_shift_right,
    )
    nc.vector.tensor_scalar(out=vi, in0=vi, scalar1=0x5F3759DF, scalar2=-1,
                            op0=ALU.subtract, op1=ALU.mult)
    r = pool.tile([pdim, n], F32, tag=f"{tag}_r", name=f"{tag}_r")
    nc.vector.tensor_copy(out=r, in_=vi.bitcast(F32))
    h = pool.tile([pdim, n], F32, tag=f"{tag}_h", name=f"{tag}_h")
    for _ in range(3):
        nc.vector.tensor_mul(out=h, in0=r, in1=r)
        nc.vector.tensor_mul(out=h, in0=h, in1=v)
        nc.vector.tensor_scalar(
            out=h, in0=h, scalar1=-0.5, scalar2=1.5, op0=ALU.mult, op1=ALU.add
        )
        nc.vector.tensor_mul(out=r, in0=r, in1=h)
    return r


def _build():
    nc = bacc.Bacc(
        "TRN2", target_bir_lowering=False, debug=False, num_devices=NCORES
    )
    x_d = nc.dram_tensor("x", [BL, C, T], FP8, kind="ExternalInput").ap()
    cpack_d = nc.dram_tensor("cpack", [P, CPB], mybir.dt.uint8,
                             kind="ExternalInput").ap()
    out_d = nc.dram_tensor("out", [BL, C, T], FP8, kind="ExternalOutput").ap()

    with tile.TileContext(nc) as tc:
        with ExitStack() as ctx:
            _emit(ctx, tc, nc, x_d, out_d, cpack_d)
    nc.compile()
    return nc


def _emit(ctx, tc, nc, x_d, out_d, cpack_d):
    const = ctx.enter_context(tc.tile_pool(name="const", bufs=1))
    xb_p = ctx.enter_context(tc.tile_pool(name="xb", bufs=4))
    y_p = ctx.enter_context(tc.tile_pool(name="y", bufs=2))
    ysq_p = ctx.enter_context(tc.tile_pool(name="ysq", bufs=1))
    ypk_p = ctx.enter_context(tc.tile_pool(name="ypk", bufs=2))
    tmp_p = ctx.enter_context(tc.tile_pool(name="tmp", bufs=2))
    hid_p = ctx.enter_context(tc.tile_pool(name="hid", bufs=2))
    sm_p = ctx.enter_context(tc.tile_pool(name="sm", bufs=2))
    row_p = ctx.enter_context(tc.tile_pool(name="row", bufs=1))
    w2s_p = ctx.enter_context(tc.tile_pool(name="w2s", bufs=2))
    ob_p = ctx.enter_context(tc.tile_pool(name="ob", bufs=3))

    dw_ps = ctx.enter_context(tc.tile_pool(name="dwps", bufs=2, space="PSUM"))
    st_ps = ctx.enter_context(tc.tile_pool(name="stps", bufs=2, space="PSUM"))
    mm_ps = ctx.enter_context(tc.tile_pool(name="mmps", bufs=2, space="PSUM"))
    rep_ps = ctx.enter_context(tc.tile_pool(name="repps", bufs=2, space="PSUM"))

    # ---- constants: ONE packed DMA, then bitcast slices ----
    cp = const.tile([P, CPB], mybir.dt.uint8)
    nc.sync.dma_start(out=cp, in_=cpack_d)
    cpf = cp.bitcast(F32)
    dwb_s = cpf[:, 0:2]
    b1f_s = cpf[:, 2:6]
    gam_s = cpf[:, 6:10]              # grn gamma, pre-scaled by OSCALE
    b2c_s = cpf[:, 10:12]             # pw2 bias (+W2@grn_beta), pre-scaled
    cpb = cp.bitcast(BF16)
    w2t_s = cpb[:, _BF0:_BF0 + 1024]
    ones_col = cpb[:, _BF0 + 1024:_BF0 + 1025]
    ones_row = cpb[0:1, _BF0 + 1025:_BF0 + 1025 + P]
    cp8 = cp.bitcast(FP8)

    xb = {}       # (s, cc) -> fp8 [P, XW] padded input (copies A and B)
    y8 = {}       # s -> bf16 [P, 2T]  (cc-major, raw dwconv out)
    ypks = {}     # s -> fp8 [P, 2T]  (normed, the pw1 DoubleRow operand)
    hid = {}      # s -> fp8 [P, 4T]  (hc-major)
    rows = {}     # s -> (r_row, nmr_row) bf16 [1, T]
    w2s = {}      # s -> scaled pw2 lhsT (fp8)

    def load(s):
        for cc in range(NCC):
            t = xb_p.tile([P, XW], FP8, tag="xb", name=f"xb_{s}_{cc}{_PFX[0]}")
            xb[(s, cc)] = t
            src = x_d[s, cc * P:(cc + 1) * P, :]
            nc.sync.dma_start(out=t[:, HALF:HALF + T], in_=src)
            nc.sync.dma_start(out=t[:, XBOFF + 3:XBOFF + 3 + T], in_=src)
            # halos: A = [x0 x0 x0 x0 | x | x_ x_ x_ x_ x_], B = A shifted 1
            nc.vector.tensor_copy(
                out=t[:, 0:HALF],
                in_=t[:, HALF:HALF + 1].to_broadcast((P, HALF)))
            nc.vector.tensor_copy(
                out=t[:, HALF + T:XA],
                in_=t[:, HALF + T - 1:HALF + T].to_broadcast((P, XA - HALF - T)))
            nc.vector.tensor_copy(
                out=t[:, XBOFF:XBOFF + 3],
                in_=t[:, XBOFF + 3:XBOFF + 4].to_broadcast((P, 3)))
            nc.vector.tensor_copy(
                out=t[:, XBOFF + 3 + T:XW],
                in_=t[:, XBOFF + 2 + T:XBOFF + 3 + T].to_broadcast(
                    (P, XW - XBOFF - 3 - T)))

    def ln_half(s, hf, sqt, r_row, nmr_row):
        # LN math for one T-half on compact [16,128] tiles; emitted as soon
        # as that half's stats are drained so the rep matmuls never stall.
        HL = T // 2
        s_c = sm_p.tile([16, P], BF16, tag=f"s_c{hf}", name=f"s_c_{s}_{hf}{_PFX[0]}")
        q_c = sm_p.tile([16, P], BF16, tag=f"q_c{hf}", name=f"q_c_{s}_{hf}{_PFX[0]}")
        nc.sync.dma_start(out=s_c, in_=sqt[0:1, hf * HL:(hf + 1) * HL])
        nc.sync.dma_start(out=q_c, in_=sqt[32:33, hf * HL:(hf + 1) * HL])
        mu = sm_p.tile([16, P], F32, tag=f"mu{hf}")
        nc.vector.tensor_scalar(out=mu, in0=s_c, scalar1=1.0 / C, scalar2=None,
                                op0=ALU.mult)
        var = sm_p.tile([16, P], F32, tag=f"var{hf}")
        nc.vector.tensor_mul(out=var, in0=mu, in1=mu)
        nc.vector.scalar_tensor_tensor(
            out=var, in0=q_c, scalar=1.0 / C, in1=var,
            op0=ALU.mult, op1=ALU.subtract)
        nc.vector.tensor_scalar(out=var, in0=var, scalar1=1e-5, scalar2=None,
                                op0=ALU.add)
        r = _rsqrt(nc, sm_p, var, 16, P, f"rs{hf}")
        nmr = sm_p.tile([16, P], F32, tag=f"nmr{hf}")
        nc.vector.scalar_tensor_tensor(out=nmr, in0=mu, scalar=-1.0, in1=r,
                                       op0=ALU.mult, op1=ALU.mult)
        r_bf = sm_p.tile([16, P], BF16, tag=f"r_bf{hf}")
        nc.vector.tensor_copy(out=r_bf, in_=r)
        nmr_bf = sm_p.tile([16, P], BF16, tag=f"nmr_bf{hf}")
        nc.vector.tensor_copy(out=nmr_bf, in_=nmr)
        nc.sync.dma_start(out=r_row[:, hf * HL:(hf + 1) * HL], in_=r_bf)
        nc.sync.dma_start(out=nmr_row[:, hf * HL:(hf + 1) * HL], in_=nmr_bf)

    def dw_stats(s):
        yt = y_p.tile([P, 2 * T], BF16, tag="y", name=f"y_{s}{_PFX[0]}")
        y8[s] = yt
        ypk = ypk_p.tile([P, 2 * T], FP8, tag="ypk", name=f"ypk_{s}{_PFX[0]}")
        ypks[s] = ypk
        ysq = ysq_p.tile([P, 2 * T], BF16, tag="ysq", name=f"ysq_{s}{_PFX[0]}")
        sqt = row_p.tile([33, T], BF16, tag="sqt", name=f"sqt_{s}{_PFX[0]}")
        r_row = row_p.tile([1, T], BF16, tag="r_row", name=f"r_row_{s}{_PFX[0]}")
        nmr_row = row_p.tile([1, T], BF16, tag="nmr_row",
                             name=f"nmr_row_{s}{_PFX[0]}")
        rows[s] = (r_row, nmr_row)
        for hh in range(2):
            for sb in range(NBLK // 2):
                blk = hh * (NBLK // 2) + sb
                lo = blk * 512
                for cc in range(NCC):
                    xt = xb[(s, cc)]
                    ps = dw_ps.tile([P, 512], F32, tag="dwps")
                    for p_ in range(4):
                        nc.tensor.matmul(
                            ps,
                            lhsT=_ap3(cp8, _DG + (p_ * NCC + cc) * 256, 128, 128, 1),
                            rhs=_ap3(xt, lo + 2 * p_, XBOFF, 512, 1),
                            start=(p_ == 0), stop=False, perf_mode=PM.DoubleRow)
                    nc.tensor.matmul(
                        ps, lhsT=cp8[:, _D8 + cc * P:_D8 + (cc + 1) * P],
                        rhs=xt[:, lo + 8:lo + 8 + 512],
                        start=False, stop=True)
                    # drain psum + dw bias -> y bf16; split across ACT and
                    # DVE (psum reads are column-rate-bound on both)
                    if cc == 0:
                        nc.scalar.activation(
                            out=yt[:, cc * T + lo:cc * T + lo + 512], in_=ps,
                            func=AF.Identity, bias=dwb_s[:, cc:cc + 1],
                            scale=1.0)
                    else:
                        nc.vector.tensor_scalar(
                            out=yt[:, cc * T + lo:cc * T + lo + 512], in0=ps,
                            scalar1=dwb_s[:, cc:cc + 1], scalar2=None,
                            op0=ALU.add)
                for cc in range(NCC):
                    nc.vector.tensor_mul(
                        out=ysq[:, cc * T + lo:cc * T + lo + 512],
                        in0=yt[:, cc * T + lo:cc * T + lo + 512],
                        in1=yt[:, cc * T + lo:cc * T + lo + 512])
                # S and Q chains in different PE column groups -> they run
                # concurrently (DoubleRow rejects 1-partition dst, so plain
                # fp8 matmuls per cc chunk; lhsT = ones16 col 0)
                st2 = st_ps.tile([64, 512], F32, tag="stps",
                                 name=f"st2_{s}_{blk}{_PFX[0]}")
                for cc in range(NCC):
                    nc.tensor.matmul(st2[0:1, :], lhsT=ones_col,
                                     rhs=yt[:, cc * T + lo:cc * T + lo + 512],
                                     start=(cc == 0), stop=(cc == NCC - 1),
                                     tile_position=(0, 0),
                                     skip_group_check=True)
                    nc.tensor.matmul(st2[32:33, :], lhsT=ones_col,
                                     rhs=ysq[:, cc * T + lo:cc * T + lo + 512],
                                     start=(cc == 0), stop=(cc == NCC - 1),
                                     tile_position=(0, 32),
                                     skip_group_check=True)
                # one drain covers S (row 0) and Q (row 32); psum-op cost
                # is column-rate-bound so 33 partitions ride along free
                nc.vector.tensor_copy(out=sqt[:, lo:lo + 512],
                                      in_=st2[0:33, :])
            ln_half(s, hh, sqt, r_row, nmr_row)

    def norm(s):
        # ypk = y*r + nmr (fp8, the DoubleRow pw1 operand); broadcasts via
        # PSUM-resident rep matmuls
        r_row, nmr_row = rows[s]
        yt = y8[s]
        ypk = ypks[s]
        for blk in range(NBLK):
            lo = blk * 512
            r_ps = rep_ps.tile([P, 512], F32, tag="repps")
            nc.tensor.matmul(r_ps, lhsT=ones_row, rhs=r_row[:, lo:lo + 512],
                             start=True, stop=True)
            n_ps = rep_ps.tile([P, 512], F32, tag="repps")
            nc.tensor.matmul(n_ps, lhsT=ones_row, rhs=nmr_row[:, lo:lo + 512],
                             start=True, stop=True)
            for cc in range(NCC):
                ysl = yt[:, cc * T + lo:cc * T + lo + 512]
                tm = tmp_p.tile([P, 512], BF16, tag="tmp")
                nc.vector.tensor_mul(out=tm, in0=ysl, in1=r_ps)
                nc.vector.tensor_add(out=ypk[:, cc * T + lo:cc * T + lo + 512],
                                     in0=tm, in1=n_ps)

    gx2s = {}

    def pw1(s):
        ht = hid_p.tile([P, 4 * T], FP8, tag="hid", name=f"hid_{s}{_PFX[0]}")
        hid[s] = ht
        yt = y8[s]
        ypk = ypks[s]
        for hc in range(NHC):
            for blk in range(NBLK):
                lo = blk * 512
                ps = mm_ps.tile([P, 512], F32, tag="mmps")
                nc.tensor.matmul(
                    ps, lhsT=_ap3(cp8, _W1 + hc * P, 512, 128, 1),
                    rhs=_ap3(ypk, lo, T, 512, 1),
                    start=True, stop=True, perf_mode=PM.DoubleRow)
                nc.scalar.activation(
                    out=ht[:, hc * T + lo:hc * T + lo + 512],
                    in_=ps, func=AF.Gelu, bias=b1f_s[:, hc:hc + 1], scale=1.0)
        # GRN square+accum on ACT; y8(s) is dead after pw1 -> scratch
        gx2 = sm_p.tile([P, NHC], F32, tag="gx2", name=f"gx2_{s}{_PFX[0]}")
        gx2s[s] = gx2
        for hc in range(NHC):
            if s == BL - 1 and hc >= 2:
                sq = yt[:, 0:T].bitcast(BF16)
                nc.vector.tensor_mul(out=sq, in0=ht[:, hc * T:(hc + 1) * T],
                                     in1=ht[:, hc * T:(hc + 1) * T])
                nc.vector.tensor_reduce(out=gx2[:, hc:hc + 1], in_=sq,
                                        axis=mybir.AxisListType.X, op=ALU.add)
            else:
                nc.scalar.activation(out=yt[:, 0:T],
                                     in_=ht[:, hc * T:(hc + 1) * T],
                                     func=AF.Square,
                                     accum_out=gx2[:, hc:hc + 1])

    def grn(s):
        gx2 = gx2s[s]
        gx2f = sm_p.tile([P, NHC], F32, tag="gx2f")
        nc.vector.tensor_scalar(out=gx2f, in0=gx2, scalar1=1e-30, scalar2=None,
                                op0=ALU.add)
        rg = _rsqrt(nc, sm_p, gx2f, P, NHC, "rg")
        gx = sm_p.tile([P, NHC], F32, tag="gx")
        nc.vector.tensor_mul(out=gx, in0=gx2f, in1=rg)      # gx = sqrt(gx2)
        gx_bf = sm_p.tile([P, NHC], BF16, tag="gx_bf")
        nc.vector.tensor_copy(out=gx_bf, in_=gx)
        # mean over all H=512 channels: ones-matmul -> [1,4] -> reduce
        gt_ps = st_ps.tile([1, NHC], F32, tag="stps", name=f"gt_{s}{_PFX[0]}")
        nc.tensor.matmul(gt_ps, lhsT=ones_col, rhs=gx_bf,
                         start=True, stop=True)
        g_row = sm_p.tile([1, NHC], F32, tag="g_row")
        nc.vector.tensor_copy(out=g_row, in_=gt_ps)
        tot = sm_p.tile([1, 1], F32, tag="tot")
        nc.vector.tensor_reduce(out=tot, in_=g_row, axis=mybir.AxisListType.X,
                                op=ALU.add)
        nc.vector.tensor_scalar(out=tot, in0=tot, scalar1=1.0 / H,
                                scalar2=1e-6, op0=ALU.mult, op1=ALU.add)
        rm_row = sm_p.tile([1, 1], F32, tag="rm_row")
        nc.vector.reciprocal(out=rm_row, in_=tot)
        rm_bf = sm_p.tile([1, 1], BF16, tag="rm_bf")
        nc.vector.tensor_copy(out=rm_bf, in_=rm_row)
        rm_ps = st_ps.tile([P, 1], F32, tag="stps", name=f"rm_{s}{_PFX[0]}")
        nc.tensor.matmul(rm_ps, lhsT=ones_row, rhs=rm_bf,
                         start=True, stop=True)
        rm = sm_p.tile([P, 1], F32, tag="rm")
        nc.vector.tensor_copy(out=rm, in_=rm_ps)
        # a = OSCALE * (gamma*nx + 1); gamma arrives pre-scaled by OSCALE
        a = sm_p.tile([P, NHC], F32, tag="a")
        nc.vector.tensor_scalar(out=a, in0=gx, scalar1=rm, scalar2=None,
                                op0=ALU.mult)
        nc.vector.scalar_tensor_tensor(out=a, in0=a, scalar=1.0, in1=gam_s,
                                       op0=ALU.bypass, op1=ALU.mult)
        nc.vector.tensor_scalar(out=a, in0=a, scalar1=OSCALE, scalar2=None,
                                op0=ALU.add)
        w2s[s] = w2s_p.tile([P, NHC * C], FP8, tag="w2s", name=f"w2s_{s}{_PFX[0]}")
        for hc in range(NHC):
            nc.vector.tensor_scalar(
                out=w2s[s][:, hc * C:(hc + 1) * C],
                in0=w2t_s[:, hc * C:(hc + 1) * C],
                scalar1=a[:, hc:hc + 1], scalar2=None, op0=ALU.mult)

    def pw2(s, last=False):
        ht = hid[s]
        for cc in range(NCC):
            for ob_i in range(2):          # two [P, 2048] fp8 out tiles per cc
                ob = ob_p.tile([P, HT], FP8, tag="ob")
                for sub in range(4):
                    lo = (ob_i * 4 + sub) * 512
                    ps = mm_ps.tile([P, 512], F32, tag="mmps")
                    for j in range(2):     # hc pairs (0,1) and (2,3)
                        nc.tensor.matmul(
                            ps,
                            lhsT=_ap3(w2s[s], (2 * j) * C + cc * P, C, 128, 1),
                            rhs=_ap3(ht, (2 * j) * T + lo, T, 512, 1),
                            start=(j == 0), stop=(j == 1),
                            perf_mode=PM.DoubleRow)
                    # drain: fp8 out = psum + bias2 (scaled by OSCALE); on the
                    # final sample nothing overlaps, so split ACT/DVE
                    if last and sub % 2 == 1:
                        nc.vector.tensor_scalar(
                            out=ob[:, sub * 512:(sub + 1) * 512], in0=ps,
                            scalar1=b2c_s[:, cc:cc + 1], scalar2=None,
                            op0=ALU.add)
                    else:
                        nc.scalar.activation(
                            out=ob[:, sub * 512:(sub + 1) * 512], in_=ps,
                            func=AF.Identity, bias=b2c_s[:, cc:cc + 1],
                            scale=1.0)
                nc.sync.dma_start(
                    out=out_d[s, cc * P:(cc + 1) * P,
                              ob_i * HT:(ob_i + 1) * HT],
                    in_=ob)

    # deferred-GRN pipeline: iter s runs dw+stats(s) / grn+pw2(s-1) /
    # norm+pw1(s); gelu+square ACT tails of pw1(s) overlap dw(s+1), giving
    # the GRN chain a full iteration of slack before pw2(s) needs w2s.
    for rp in range(_REPEAT):
        _PFX[0] = f"_rp{rp}" if _REPEAT > 1 else ""
        load(0)
        for s in range(BL):
            if s + 1 < BL:
                load(s + 1)
            dw_stats(s)
            if s >= 1:
                grn(s - 1)
                pw2(s - 1)
            norm(s)
            pw1(s)
        grn(BL - 1)
        pw2(BL - 1, last=True)


def _prep_inputs(inputs):
    x = np.asarray(inputs["x"], np.float32)
    dw_w = np.asarray(inputs["dw_w"], np.float32)      # (C,1,K)
    dw_b = np.asarray(inputs["dw_b"], np.float32)
    ln_w = np.asarray(inputs["ln_w"], np.float32)
    ln_b = np.asarray(inputs["ln_b"], np.float32)
    pw1_w = np.asarray(inputs["pw1_w"], np.float32)    # (H,C)
    pw1_b = np.asarray(inputs["pw1_b"], np.float32)
    gg = np.asarray(inputs["grn_gamma"], np.float32)
    gb = np.asarray(inputs["grn_beta"], np.float32)
    pw2_w = np.asarray(inputs["pw2_w"], np.float32)    # (C,H)
    pw2_b = np.asarray(inputs["pw2_b"], np.float32)

    dwb = dw_b.reshape(NCC, P).T.copy()
    b1f = (pw1_b + pw1_w @ ln_b).reshape(NHC, P).T.copy()
    gam = (gg * OSCALE).reshape(NHC, P).T.copy()
    b2c = ((pw2_b + pw2_w @ gb) * OSCALE).reshape(NCC, P).T.copy()

    w2t = np.zeros((P, NHC * C), BF)
    for hc in range(NHC):
        w2t[:, hc * C:(hc + 1) * C] = \
            pw2_w[:, hc * P:(hc + 1) * P].T.astype(BF)
    onescol = np.ones((P, 1), BF)
    onesrow_blk = np.zeros((P, P), BF)
    onesrow_blk[0, :] = 1.0

    # fp8 block: dwconv diag pairs, tap-8 diags, w1pk, ones16
    dgpk = np.zeros((P, 2048), F8)
    for p_ in range(4):
        for cc in range(NCC):
            base = (p_ * NCC + cc) * 256
            for j in range(2):
                k = 2 * p_ + j
                dg = np.zeros((P, P), np.float32)
                np.fill_diagonal(dg, dw_w[cc * P:(cc + 1) * P, 0, k])
                dgpk[:, base + j * P:base + (j + 1) * P] = dg.astype(F8)
    d8 = np.zeros((P, 2 * P), F8)
    for cc in range(NCC):
        dg = np.zeros((P, P), np.float32)
        np.fill_diagonal(dg, dw_w[cc * P:(cc + 1) * P, 0, 8])
        d8[:, cc * P:(cc + 1) * P] = dg.astype(F8)
    w1f = pw1_w * ln_w[None, :]                        # (H,C)
    w1pk = np.zeros((P, 1024), F8)
    for cc in range(NCC):
        for hc in range(NHC):
            w1pk[:, cc * 512 + hc * P:cc * 512 + (hc + 1) * P] = \
                w1f[hc * P:(hc + 1) * P, cc * P:(cc + 1) * P].T.astype(F8)
    ones16 = np.zeros((P, 17), F8)
    ones16[:, 0] = 1.0
    ones16[:, 16] = 1.0

    cpack = np.concatenate([
        dwb.view(np.uint8), b1f.view(np.uint8), gam.view(np.uint8),
        b2c.view(np.uint8),
        w2t.view(np.uint8), onescol.view(np.uint8), onesrow_blk.view(np.uint8),
        dgpk.view(np.uint8), d8.view(np.uint8), w1pk.view(np.uint8),
        ones16.view(np.uint8)], axis=1)
    pad = CPB - cpack.shape[1]
    if pad:
        cpack = np.concatenate([cpack, np.zeros((P, pad), np.uint8)], axis=1)
    assert cpack.shape == (P, CPB), cpack.shape
    x8 = x.astype(F8)
    common = {"cpack": np.ascontiguousarray(cpack)}
    in_maps = []
    for i in range(NCORES):
        m = dict(common)
        m["x"] = x8[i * BL:(i + 1) * BL]
        in_maps.append(m)
    return in_maps, x


def kernel(**inputs):
    if "nc" not in _CACHE:
        _CACHE["nc"] = _build()
    nc = _CACHE["nc"]
    in_maps, x = _prep_inputs(inputs)
    res = run_bass_kernel_spmd(nc, in_maps, core_ids=list(range(NCORES)),
                               **_CACHE.get("run_kwargs", {}))
    _CACHE["last_result"] = res
    y8 = np.concatenate([np.asarray(res.results[i]["out"])
                         for i in range(NCORES)], axis=0)
    return x + y8.astype(np.float32) * (1.0 / OSCALE)


# revision 19
# speedup vs baseline: 1.1005x; 1.0309x over previous
"""ConvNeXtV2 block (B=32, C=256, T=4096, K=9, H=512) on 8 trn2 cores.

Data-parallel over batch: 4 samples per core, no collectives.

v6 design notes (v5 was 574us, DVE-bound at 80% by the STT dwconv):
- fp8(e4m3) everywhere on the matmul path, exploiting DoubleRow perf
  mode (2 fp8 contraction rows per PE cell):
  * dwconv: back on the PE as 4 DoubleRow diag-pair matmuls + 1 plain
    fp8 matmul per (cc, 512-block) -- 5 column streams instead of 9.
    The tap pair (k, k+1) needs rhs rows at element stride 1, which the
    ifmap AP rejects (stride-1 hard-hangs the PE), so x is DMA'd TWICE
    into one SBUF tile: copy A (padded x) at 0 and copy B (x shifted by
    one) at a 16-aligned offset; the pair stride is then 4112. The
    extra HBM read is free - DMA is nowhere near roofline.
  * pw1: one DoubleRow matmul contracts all of C=256 per (hc, blk).
  * pw2: two DoubleRow matmuls contract H=512 per (cc, blk).
  * LN stats: S and Q each via one DoubleRow ones-pair matmul per block
    (lhsT = [1|...15 zeros...|1] fp8 row pair at stride 16).
- y (dwconv out) kept as one [128, 2T] fp8 tile per sample; the LN
  "-mu*r" rank-1 fold is gone: norm computes y = y*r + nmr with two
  tensor_tensor ops against PSUM-broadcast rows (r and nmr).
- x input fp8, block output y*8 in fp8, residual added on host in f32
  (unchanged from v5; I/O is 67MB/call vs 268MB for f32 in/out).
- no gpsimd/SWDGE anywhere; all DMAs are HWDGE (sync).
Host pre-folds ln_w/ln_b into pw1, grn_beta and the fp8 x8 scale into
the pw2 bias/gamma; dwconv taps and pw1 weights are pre-quantized to
fp8 on the host (LN washes the ~4% dwconv error; the residual dilutes
everything by ~12x).
"""

from contextlib import ExitStack

import ml_dtypes
import numpy as np

import concourse.bass as bass
import concourse.mybir as mybir
import concourse.tile as tile
from concourse import bacc
from concourse.bass_utils import run_bass_kernel_spmd

B, C, T, K, H = 32, 256, 4096, 9, 512
NCORES = 8
BL = B // NCORES          # samples per core
P = 128
NCC = C // P              # 2 channel chunks
NHC = H // P              # 4 hidden chunks
NBLK = T // 512           # 8 column blocks of 512
HALF = K // 2             # 4
HT = T // 2               # 2048 columns per half-row
F32 = mybir.dt.float32
BF16 = mybir.dt.bfloat16
FP8 = mybir.dt.float8e4
I32 = mybir.dt.int32
BF = ml_dtypes.bfloat16
F8 = ml_dtypes.float8_e4m3
ALU = mybir.AluOpType
AF = mybir.ActivationFunctionType
PM = mybir.MatmulPerfMode

OSCALE = 8.0              # block output scaled by 8 before the fp8 write

_CACHE = {}
_REPEAT = 1    # timing-only knob: emit the whole pipeline N times in one NEFF
_PFX = [""]    # tile-name suffix per repeat (names must be unique)

# xbw layout: copy A (padded x, 4105 elems) at 0, copy B (=A shifted by
# one element) at XBOFF (16-aligned so the DoubleRow pair stride is legal)
XA = 4105
XBOFF = 4112
XW = XBOFF + 4104

# cpack layout
_NF32 = 2 + 4 + 4 + 2                          # dwb, b1f, gam8, b2c8
_BF0 = _NF32 * 2                               # bf16 elem offset (=24)
_NBF = 1024 + 1 + 128                          # w2t, ones_col, ones_row
_F80 = _NF32 * 4 + _NBF * 2                    # fp8 byte offset (=2354)
_DG = _F80                                     # diag pairs: 4 pairs x 2cc x 256
_D8 = _DG + 2048                               # tap-8 diags: 2cc x 128
_W1 = _D8 + 256                                # w1pk: 1024
_O16 = _W1 + 1024                              # ones16: 17
CPB = _O16 + 17
CPB += (-CPB) % 4


def _ap3(t, off, s1, n2, s2):
    """[128, 2, n2] AP over tile t at element offset off (pair stride s1,
    inner stride s2) -- the 3D form DoubleRow matmuls consume."""
    v = t[:, off:off + 1]
    c = v.copy()
    pstride = list(c.ap[0])
    c.ap[:] = [pstride, [s1, 2], [s2, n2]]
    return c


def _rsqrt(nc, pool, v, pdim, n, tag):
    """Newton rsqrt on DVE for a small [pdim, n] f32 tile (avoids the ACT
    sqrt table set; gelu set stays resident)."""
    vi = pool.tile([pdim, n], I32, tag=f"{tag}_i", name=f"{tag}_i")
    nc.vector.tensor_scalar(
        out=vi, in0=v.bitcast(I32), scalar1=1, scalar2=None,
        op0=ALU.logical_shift_right,
    )
    nc.vector.tensor_scalar(out=vi, in0=vi, scalar1=0x5F3759DF, scalar2=-1,
                            op0=ALU.subtract, op1=ALU.mult)
    r = pool.tile([pdim, n], F32, tag=f"{tag}_r", name=f"{tag}_r")
    nc.vector.tensor_copy(out=r, in_=vi.bitcast(F32))
    h = pool.tile([pdim, n], F32, tag=f"{tag}_h", name=f"{tag}_h")
    for _ in range(3):
        nc.vector.tensor_mul(out=h, in0=r, in1=r)
        nc.vector.tensor_mul(out=h, in0=h, in1=v)
        nc.vector.tensor_scalar(
            out=h, in0=h, scalar1=-0.5, scalar2=1.5, op0=ALU.mult, op1=ALU.add
        )
        nc.vector.tensor_mul(out=r, in0=r, in1=h)
    return r


def _build():
    nc = bacc.Bacc(
        "TRN2", target_bir_lowering=False, debug=False, num_devices=NCORES
    )
    x_d = nc.dram_tensor("x", [BL, C, T], FP8, kind="ExternalInput").ap()
    cpack_d = nc.dram_tensor("cpack", [P, CPB], mybir.dt.uint8,
                             kind="ExternalInput").ap()
    out_d = nc.dram_tensor("out", [BL, C, T], FP8, kind="ExternalOutput").ap()

    with tile.TileContext(nc) as tc:
        with ExitStack() as ctx:
            _emit(ctx, tc, nc, x_d, out_d, cpack_d)
    nc.compile()
    return nc


def _emit(ctx, tc, nc, x_d, out_d, cpack_d):
    const = ctx.enter_context(tc.tile_pool(name="const", bufs=1))
    xb_p = ctx.enter_context(tc.tile_pool(name="xb", bufs=4))
    y_p = ctx.enter_context(tc.tile_pool(name="y", bufs=2))
    ysq_p = ctx.enter_context(tc.tile_pool(name="ysq", bufs=1))
    ypk_p = ctx.enter_context(tc.tile_pool(name="ypk", bufs=2))
    tmp_p = ctx.enter_context(tc.tile_pool(name="tmp", bufs=2))
    hid_p = ctx.enter_context(tc.tile_pool(name="hid", bufs=2))
    sm_p = ctx.enter_context(tc.tile_pool(name="sm", bufs=2))
    row_p = ctx.enter_context(tc.tile_pool(name="row", bufs=1))
    w2s_p = ctx.enter_context(tc.tile_pool(name="w2s", bufs=2))
    ob_p = ctx.enter_context(tc.tile_pool(name="ob", bufs=3))

    dw_ps = ctx.enter_context(tc.tile_pool(name="dwps", bufs=2, space="PSUM"))
    st_ps = ctx.enter_context(tc.tile_pool(name="stps", bufs=1, space="PSUM"))
    mm_ps = ctx.enter_context(tc.tile_pool(name="mmps", bufs=3, space="PSUM"))
    rep_ps = ctx.enter_context(tc.tile_pool(name="repps", bufs=2, space="PSUM"))

    # ---- constants: ONE packed DMA, then bitcast slices ----
    cp = const.tile([P, CPB], mybir.dt.uint8)
    nc.sync.dma_start(out=cp, in_=cpack_d)
    cpf = cp.bitcast(F32)
    dwb_s = cpf[:, 0:2]
    b1f_s = cpf[:, 2:6]
    gam_s = cpf[:, 6:10]              # grn gamma, pre-scaled by OSCALE
    b2c_s = cpf[:, 10:12]             # pw2 bias (+W2@grn_beta), pre-scaled
    cpb = cp.bitcast(BF16)
    w2t_s = cpb[:, _BF0:_BF0 + 1024]
    ones_col = cpb[:, _BF0 + 1024:_BF0 + 1025]
    ones_row = cpb[0:1, _BF0 + 1025:_BF0 + 1025 + P]
    cp8 = cp.bitcast(FP8)

    xb = {}       # (s, cc) -> fp8 [P, XW] padded input (copies A and B)
    y8 = {}       # s -> bf16 [P, 2T]  (cc-major, raw dwconv out)
    ypks = {}     # s -> fp8 [P, 2T]  (normed, the pw1 DoubleRow operand)
    hid = {}      # s -> fp8 [P, 4T]  (hc-major)
    rows = {}     # s -> (r_row, nmr_row) bf16 [1, T]
    w2s = {}      # s -> scaled pw2 lhsT (fp8)

    def load(s):
        for cc in range(NCC):
            t = xb_p.tile([P, XW], FP8, tag="xb", name=f"xb_{s}_{cc}{_PFX[0]}")
            xb[(s, cc)] = t
            src = x_d[s, cc * P:(cc + 1) * P, :]
            nc.sync.dma_start(out=t[:, HALF:HALF + T], in_=src)
            nc.sync.dma_start(out=t[:, XBOFF + 3:XBOFF + 3 + T], in_=src)
            # halos: A = [x0 x0 x0 x0 | x | x_ x_ x_ x_ x_], B = A shifted 1
            nc.vector.tensor_copy(
                out=t[:, 0:HALF],
                in_=t[:, HALF:HALF + 1].to_broadcast((P, HALF)))
            nc.vector.tensor_copy(
                out=t[:, HALF + T:XA],
                in_=t[:, HALF + T - 1:HALF + T].to_broadcast((P, XA - HALF - T)))
            nc.vector.tensor_copy(
                out=t[:, XBOFF:XBOFF + 3],
                in_=t[:, XBOFF + 3:XBOFF + 4].to_broadcast((P, 3)))
            nc.vector.tensor_copy(
                out=t[:, XBOFF + 3 + T:XW],
                in_=t[:, XBOFF + 2 + T:XBOFF + 3 + T].to_broadcast(
                    (P, XW - XBOFF - 3 - T)))

    def ln_half(s, hf, sqt, r_row, nmr_row):
        # LN math for one T-half on compact [16,128] tiles; emitted as soon
        # as that half's stats are drained so the rep matmuls never stall.
        HL = T // 2
        s_c = sm_p.tile([16, P], BF16, tag=f"s_c{hf}", name=f"s_c_{s}_{hf}{_PFX[0]}")
        q_c = sm_p.tile([16, P], BF16, tag=f"q_c{hf}", name=f"q_c_{s}_{hf}{_PFX[0]}")
        nc.sync.dma_start(out=s_c, in_=sqt[0:1, hf * HL:(hf + 1) * HL])
        nc.sync.dma_start(out=q_c, in_=sqt[32:33, hf * HL:(hf + 1) * HL])
        mu = sm_p.tile([16, P], F32, tag=f"mu{hf}")
        nc.vector.tensor_scalar(out=mu, in0=s_c, scalar1=1.0 / C, scalar2=None,
                                op0=ALU.mult)
        var = sm_p.tile([16, P], F32, tag=f"var{hf}")
        nc.vector.tensor_mul(out=var, in0=mu, in1=mu)
        nc.vector.scalar_tensor_tensor(
            out=var, in0=q_c, scalar=1.0 / C, in1=var,
            op0=ALU.mult, op1=ALU.subtract)
        nc.vector.tensor_scalar(out=var, in0=var, scalar1=1e-5, scalar2=None,
                                op0=ALU.add)
        r = _rsqrt(nc, sm_p, var, 16, P, f"rs{hf}")
        nmr = sm_p.tile([16, P], F32, tag=f"nmr{hf}")
        nc.vector.scalar_tensor_tensor(out=nmr, in0=mu, scalar=-1.0, in1=r,
                                       op0=ALU.mult, op1=ALU.mult)
        r_bf = sm_p.tile([16, P], BF16, tag=f"r_bf{hf}")
        nc.vector.tensor_copy(out=r_bf, in_=r)
        nmr_bf = sm_p.tile([16, P], BF16, tag=f"nmr_bf{hf}")
        nc.vector.tensor_copy(out=nmr_bf, in_=nmr)
        nc.sync.dma_start(out=r_row[:, hf * HL:(hf + 1) * HL], in_=r_bf)
        nc.sync.dma_start(out=nmr_row[:, hf * HL:(hf + 1) * HL], in_=nmr_bf)

    def dw_stats(s):
        yt = y_p.tile([P, 2 * T], BF16, tag="y", name=f"y_{s}{_PFX[0]}")
        y8[s] = yt
        ypk = ypk_p.tile([P, 2 * T], FP8, tag="ypk", name=f"ypk_{s}{_PFX[0]}")
        ypks[s] = ypk
        ysq = ysq_p.tile([P, 2 * T], BF16, tag="ysq", name=f"ysq_{s}{_PFX[0]}")
        sqt = row_p.tile([33, T], BF16, tag="sqt", name=f"sqt_{s}{_PFX[0]}")
        r_row = row_p.tile([1, T], BF16, tag="r_row", name=f"r_row_{s}{_PFX[0]}")
        nmr_row = row_p.tile([1, T], BF16, tag="nmr_row",
                             name=f"nmr_row_{s}{_PFX[0]}")
        rows[s] = (r_row, nmr_row)
        for hh in range(2):
            for sb in range(NBLK // 2):
                blk = hh * (NBLK // 2) + sb
                lo = blk * 512
                for cc in range(NCC):
                    xt = xb[(s, cc)]
                    ps = dw_ps.tile([P, 512], F32, tag="dwps")
                    for p_ in range(4):
                        nc.tensor.matmul(
                            ps,
                            lhsT=_ap3(cp8, _DG + (p_ * NCC + cc) * 256, 128, 128, 1),
                            rhs=_ap3(xt, lo + 2 * p_, XBOFF, 512, 1),
                            start=(p_ == 0), stop=False, perf_mode=PM.DoubleRow)
                    nc.tensor.matmul(
                        ps, lhsT=cp8[:, _D8 + cc * P:_D8 + (cc + 1) * P],
                        rhs=xt[:, lo + 8:lo + 8 + 512],
                        start=False, stop=True)
                    # drain psum + dw bias -> y bf16; split across ACT and
                    # DVE (psum reads are column-rate-bound on both)
                    if cc == 0:
                        nc.scalar.activation(
                            out=yt[:, cc * T + lo:cc * T + lo + 512], in_=ps,
                            func=AF.Identity, bias=dwb_s[:, cc:cc + 1],
                            scale=1.0)
                    else:
                        nc.vector.tensor_scalar(
                            out=yt[:, cc * T + lo:cc * T + lo + 512], in0=ps,
                            scalar1=dwb_s[:, cc:cc + 1], scalar2=None,
                            op0=ALU.add)
                for cc in range(NCC):
                    nc.vector.tensor_mul(
                        out=ysq[:, cc * T + lo:cc * T + lo + 512],
                        in0=yt[:, cc * T + lo:cc * T + lo + 512],
                        in1=yt[:, cc * T + lo:cc * T + lo + 512])
                # S and Q chains in different PE column groups -> they run
                # concurrently (DoubleRow rejects 1-partition dst, so plain
                # fp8 matmuls per cc chunk; lhsT = ones16 col 0)
                st2 = st_ps.tile([64, 512], F32, tag="stps",
                                 name=f"st2_{s}_{blk}{_PFX[0]}")
                for cc in range(NCC):
                    nc.tensor.matmul(st2[0:1, :], lhsT=ones_col,
                                     rhs=yt[:, cc * T + lo:cc * T + lo + 512],
                                     start=(cc == 0), stop=(cc == NCC - 1),
                                     tile_position=(0, 0),
                                     skip_group_check=True)
                    nc.tensor.matmul(st2[32:33, :], lhsT=ones_col,
                                     rhs=ysq[:, cc * T + lo:cc * T + lo + 512],
                                     start=(cc == 0), stop=(cc == NCC - 1),
                                     tile_position=(0, 32),
                                     skip_group_check=True)
                # one drain covers S (row 0) and Q (row 32); psum-op cost
                # is column-rate-bound so 33 partitions ride along free
                nc.vector.tensor_copy(out=sqt[:, lo:lo + 512],
                                      in_=st2[0:33, :])
            ln_half(s, hh, sqt, r_row, nmr_row)
            norm_half(s, hh)

    def norm_half(s, hh):
        # ypk = y*r + nmr (fp8, the DoubleRow pw1 operand); broadcasts via
        # PSUM-resident rep matmuls; emitted per T-half right after that
        # half's LN rows land so DVE fills the other half's stats window
        r_row, nmr_row = rows[s]
        yt = y8[s]
        ypk = ypks[s]
        for blk in range(hh * (NBLK // 2), (hh + 1) * (NBLK // 2)):
            lo = blk * 512
            r_ps = rep_ps.tile([P, 512], F32, tag="repps")
            nc.tensor.matmul(r_ps, lhsT=ones_row, rhs=r_row[:, lo:lo + 512],
                             start=True, stop=True)
            n_ps = rep_ps.tile([P, 512], F32, tag="repps")
            nc.tensor.matmul(n_ps, lhsT=ones_row, rhs=nmr_row[:, lo:lo + 512],
                             start=True, stop=True)
            for cc in range(NCC):
                ysl = yt[:, cc * T + lo:cc * T + lo + 512]
                tm = tmp_p.tile([P, 512], BF16, tag="tmp")
                nc.vector.tensor_mul(out=tm, in0=ysl, in1=r_ps)
                nc.vector.tensor_add(out=ypk[:, cc * T + lo:cc * T + lo + 512],
                                     in0=tm, in1=n_ps)

    gx2s = {}

    def pw1(s):
        ht = hid_p.tile([P, 4 * T], FP8, tag="hid", name=f"hid_{s}{_PFX[0]}")
        hid[s] = ht
        yt = y8[s]
        ypk = ypks[s]
        for hc in range(NHC):
            for blk in range(NBLK):
                lo = blk * 512
                ps = mm_ps.tile([P, 512], F32, tag="mmps")
                nc.tensor.matmul(
                    ps, lhsT=_ap3(cp8, _W1 + hc * P, 512, 128, 1),
                    rhs=_ap3(ypk, lo, T, 512, 1),
                    start=True, stop=True, perf_mode=PM.DoubleRow)
                nc.scalar.activation(
                    out=ht[:, hc * T + lo:hc * T + lo + 512],
                    in_=ps, func=AF.Gelu, bias=b1f_s[:, hc:hc + 1], scale=1.0)
        # GRN square+accum on ACT; y8(s) is dead after pw1 -> scratch
        gx2 = sm_p.tile([P, NHC], F32, tag="gx2", name=f"gx2_{s}{_PFX[0]}")
        gx2s[s] = gx2
        for hc in range(NHC):
            if s == BL - 1 and hc >= 2:
                sq = yt[:, 0:T].bitcast(BF16)
                nc.vector.tensor_mul(out=sq, in0=ht[:, hc * T:(hc + 1) * T],
                                     in1=ht[:, hc * T:(hc + 1) * T])
                nc.vector.tensor_reduce(out=gx2[:, hc:hc + 1], in_=sq,
                                        axis=mybir.AxisListType.X, op=ALU.add)
            else:
                nc.scalar.activation(out=yt[:, 0:T],
                                     in_=ht[:, hc * T:(hc + 1) * T],
                                     func=AF.Square,
                                     accum_out=gx2[:, hc:hc + 1])

    def grn(s):
        gx2 = gx2s[s]
        gx2f = sm_p.tile([P, NHC], F32, tag="gx2f")
        nc.vector.tensor_scalar(out=gx2f, in0=gx2, scalar1=1e-30, scalar2=None,
                                op0=ALU.add)
        rg = _rsqrt(nc, sm_p, gx2f, P, NHC, "rg")
        gx = sm_p.tile([P, NHC], F32, tag="gx")
        nc.vector.tensor_mul(out=gx, in0=gx2f, in1=rg)      # gx = sqrt(gx2)
        gx_bf = sm_p.tile([P, NHC], BF16, tag="gx_bf")
        nc.vector.tensor_copy(out=gx_bf, in_=gx)
        # mean over all H=512 channels: ones-matmul -> [1,4] -> reduce
        gt_ps = st_ps.tile([1, NHC], F32, tag="stps", name=f"gt_{s}{_PFX[0]}")
        nc.tensor.matmul(gt_ps, lhsT=ones_col, rhs=gx_bf,
                         start=True, stop=True)
        g_row = sm_p.tile([1, NHC], F32, tag="g_row")
        nc.vector.tensor_copy(out=g_row, in_=gt_ps)
        tot = sm_p.tile([1, 1], F32, tag="tot")
        nc.vector.tensor_reduce(out=tot, in_=g_row, axis=mybir.AxisListType.X,
                                op=ALU.add)
        nc.vector.tensor_scalar(out=tot, in0=tot, scalar1=1.0 / H,
                                scalar2=1e-6, op0=ALU.mult, op1=ALU.add)
        rm_row = sm_p.tile([1, 1], F32, tag="rm_row")
        nc.vector.reciprocal(out=rm_row, in_=tot)
        rm_bf = sm_p.tile([1, 1], BF16, tag="rm_bf")
        nc.vector.tensor_copy(out=rm_bf, in_=rm_row)
        rm_ps = st_ps.tile([P, 1], F32, tag="stps", name=f"rm_{s}{_PFX[0]}")
        nc.tensor.matmul(rm_ps, lhsT=ones_row, rhs=rm_bf,
                         start=True, stop=True)
        rm = sm_p.tile([P, 1], F32, tag="rm")
        nc.vector.tensor_copy(out=rm, in_=rm_ps)
        # a = OSCALE * (gamma*nx + 1); gamma arrives pre-scaled by OSCALE
        a = sm_p.tile([P, NHC], F32, tag="a")
        nc.vector.tensor_scalar(out=a, in0=gx, scalar1=rm, scalar2=None,
                                op0=ALU.mult)
        nc.vector.scalar_tensor_tensor(out=a, in0=a, scalar=1.0, in1=gam_s,
                                       op0=ALU.bypass, op1=ALU.mult)
        nc.vector.tensor_scalar(out=a, in0=a, scalar1=OSCALE, scalar2=None,
                                op0=ALU.add)
        w2s[s] = w2s_p.tile([P, NHC * C], FP8, tag="w2s", name=f"w2s_{s}{_PFX[0]}")
        for hc in range(NHC):
            nc.vector.tensor_scalar(
                out=w2s[s][:, hc * C:(hc + 1) * C],
                in0=w2t_s[:, hc * C:(hc + 1) * C],
                scalar1=a[:, hc:hc + 1], scalar2=None, op0=ALU.mult)

    def pw2(s, last=False):
        ht = hid[s]
        for cc in range(NCC):
            for ob_i in range(2):          # two [P, 2048] fp8 out tiles per cc
                ob = ob_p.tile([P, HT], FP8, tag="ob")
                for sub in range(4):
                    lo = (ob_i * 4 + sub) * 512
                    ps = mm_ps.tile([P, 512], F32, tag="mmps")
                    for j in range(2):     # hc pairs (0,1) and (2,3)
                        nc.tensor.matmul(
                            ps,
                            lhsT=_ap3(w2s[s], (2 * j) * C + cc * P, C, 128, 1),
                            rhs=_ap3(ht, (2 * j) * T + lo, T, 512, 1),
                            start=(j == 0), stop=(j == 1),
                            perf_mode=PM.DoubleRow)
                    # drain: fp8 out = psum + bias2 (scaled by OSCALE); on the
                    # final sample nothing overlaps, so split ACT/DVE
                    if last and sub % 2 == 1:
                        nc.vector.tensor_scalar(
                            out=ob[:, sub * 512:(sub + 1) * 512], in0=ps,
                            scalar1=b2c_s[:, cc:cc + 1], scalar2=None,
                            op0=ALU.add)
                    else:
                        nc.scalar.activation(
                            out=ob[:, sub * 512:(sub + 1) * 512], in_=ps,
                            func=AF.Identity, bias=b2c_s[:, cc:cc + 1],
                            scale=1.0)
                nc.sync.dma_start(
                    out=out_d[s, cc * P:(cc + 1) * P,
                              ob_i * HT:(ob_i + 1) * HT],
                    in_=ob)

    # deferred-GRN pipeline: iter s runs dw+stats(s) / grn+pw2(s-1) /
    # norm+pw1(s); gelu+square ACT tails of pw1(s) overlap dw(s+1), giving
    # the GRN chain a full iteration of slack before pw2(s) needs w2s.
    for rp in range(_REPEAT):
        _PFX[0] = f"_rp{rp}" if _REPEAT > 1 else ""
        load(0)
        for s in range(BL):
            if s + 1 < BL:
                load(s + 1)
            dw_stats(s)
            if s >= 1:
                grn(s - 1)
                pw2(s - 1)
            pw1(s)
        grn(BL - 1)
        pw2(BL - 1, last=True)


def _prep_inputs(inputs):
    x = np.asarray(inputs["x"], np.float32)
    dw_w = np.asarray(inputs["dw_w"], np.float32)      # (C,1,K)
    dw_b = np.asarray(inputs["dw_b"], np.float32)
    ln_w = np.asarray(inputs["ln_w"], np.float32)
    ln_b = np.asarray(inputs["ln_b"], np.float32)
    pw1_w = np.asarray(inputs["pw1_w"], np.float32)    # (H,C)
    pw1_b = np.asarray(inputs["pw1_b"], np.float32)
    gg = np.asarray(inputs["grn_gamma"], np.float32)
    gb = np.asarray(inputs["grn_beta"], np.float32)
    pw2_w = np.asarray(inputs["pw2_w"], np.float32)    # (C,H)
    pw2_b = np.asarray(inputs["pw2_b"], np.float32)

    dwb = dw_b.reshape(NCC, P).T.copy()
    b1f = (pw1_b + pw1_w @ ln_b).reshape(NHC, P).T.copy()
    gam = (gg * OSCALE).reshape(NHC, P).T.copy()
    b2c = ((pw2_b + pw2_w @ gb) * OSCALE).reshape(NCC, P).T.copy()

    w2t = np.zeros((P, NHC * C), BF)
    for hc in range(NHC):
        w2t[:, hc * C:(hc + 1) * C] = \
            pw2_w[:, hc * P:(hc + 1) * P].T.astype(BF)
    onescol = np.ones((P, 1), BF)
    onesrow_blk = np.zeros((P, P), BF)
    onesrow_blk[0, :] = 1.0

    # fp8 block: dwconv diag pairs, tap-8 diags, w1pk, ones16
    dgpk = np.zeros((P, 2048), F8)
    for p_ in range(4):
        for cc in range(NCC):
            base = (p_ * NCC + cc) * 256
            for j in range(2):
                k = 2 * p_ + j
                dg = np.zeros((P, P), np.float32)
                np.fill_diagonal(dg, dw_w[cc * P:(cc + 1) * P, 0, k])
                dgpk[:, base + j * P:base + (j + 1) * P] = dg.astype(F8)
    d8 = np.zeros((P, 2 * P), F8)
    for cc in range(NCC):
        dg = np.zeros((P, P), np.float32)
        np.fill_diagonal(dg, dw_w[cc * P:(cc + 1) * P, 0, 8])
        d8[:, cc * P:(cc + 1) * P] = dg.astype(F8)
    w1f = pw1_w * ln_w[None, :]                        # (H,C)
    w1pk = np.zeros((P, 1024), F8)
    for cc in range(NCC):
        for hc in range(NHC):
            w1pk[:, cc * 512 + hc * P:cc * 512 + (hc + 1) * P] = \
                w1f[hc * P:(hc + 1) * P, cc * P:(cc + 1) * P].T.astype(F8)
    ones16 = np.zeros((P, 17), F8)
    ones16[:, 0] = 1.0
    ones16[:, 16] = 1.0

    cpack = np.concatenate([
        dwb.view(np.uint8), b1f.view(np.uint8), gam.view(np.uint8),
        b2c.view(np.uint8),
        w2t.view(np.uint8), onescol.view(np.uint8), onesrow_blk.view(np.uint8),
        dgpk.view(np.uint8), d8.view(np.uint8), w1pk.view(np.uint8),
        ones16.view(np.uint8)], axis=1)
    pad = CPB - cpack.shape[1]
    if pad:
        cpack = np.concatenate([cpack, np.zeros((P, pad), np.uint8)], axis=1)
    assert cpack.shape == (P, CPB), cpack.shape
    x8 = x.astype(F8)
    common = {"cpack": np.ascontiguousarray(cpack)}
    in_maps = []
    for i in range(NCORES):
        m = dict(common)
        m["x"] = x8[i * BL:(i + 1) * BL]
        in_maps.append(m)
    return in_maps, x


def kernel(**inputs):
    if "nc" not in _CACHE:
        _CACHE["nc"] = _build()
    nc = _CACHE["nc"]
    in_maps, x = _prep_inputs(inputs)
    res = run_bass_kernel_spmd(nc, in_maps, core_ids=list(range(NCORES)),
                               **_CACHE.get("run_kwargs", {}))
    _CACHE["last_result"] = res
    y8 = np.concatenate([np.asarray(res.results[i]["out"])
                         for i in range(NCORES)], axis=0)
    return x + y8.astype(np.float32) * (1.0 / OSCALE)


# revision 20
# speedup vs baseline: 1.1081x; 1.0069x over previous
"""ConvNeXtV2 block (B=32, C=256, T=4096, K=9, H=512) on 8 trn2 cores.

Data-parallel over batch: 4 samples per core, no collectives.

v6 design notes (v5 was 574us, DVE-bound at 80% by the STT dwconv):
- fp8(e4m3) everywhere on the matmul path, exploiting DoubleRow perf
  mode (2 fp8 contraction rows per PE cell):
  * dwconv: back on the PE as 4 DoubleRow diag-pair matmuls + 1 plain
    fp8 matmul per (cc, 512-block) -- 5 column streams instead of 9.
    The tap pair (k, k+1) needs rhs rows at element stride 1, which the
    ifmap AP rejects (stride-1 hard-hangs the PE), so x is DMA'd TWICE
    into one SBUF tile: copy A (padded x) at 0 and copy B (x shifted by
    one) at a 16-aligned offset; the pair stride is then 4112. The
    extra HBM read is free - DMA is nowhere near roofline.
  * pw1: one DoubleRow matmul contracts all of C=256 per (hc, blk).
  * pw2: two DoubleRow matmuls contract H=512 per (cc, blk).
  * LN stats: S and Q each via one DoubleRow ones-pair matmul per block
    (lhsT = [1|...15 zeros...|1] fp8 row pair at stride 16).
- y (dwconv out) kept as one [128, 2T] fp8 tile per sample; the LN
  "-mu*r" rank-1 fold is gone: norm computes y = y*r + nmr with two
  tensor_tensor ops against PSUM-broadcast rows (r and nmr).
- x input fp8, block output y*8 in fp8, residual added on host in f32
  (unchanged from v5; I/O is 67MB/call vs 268MB for f32 in/out).
- no gpsimd/SWDGE anywhere; all DMAs are HWDGE (sync).
Host pre-folds ln_w/ln_b into pw1, grn_beta and the fp8 x8 scale into
the pw2 bias/gamma; dwconv taps and pw1 weights are pre-quantized to
fp8 on the host (LN washes the ~4% dwconv error; the residual dilutes
everything by ~12x).
"""

from contextlib import ExitStack

import ml_dtypes
import numpy as np

import concourse.bass as bass
import concourse.mybir as mybir
import concourse.tile as tile
from concourse import bacc
from concourse.bass_utils import run_bass_kernel_spmd

B, C, T, K, H = 32, 256, 4096, 9, 512
NCORES = 8
BL = B // NCORES          # samples per core
P = 128
NCC = C // P              # 2 channel chunks
NHC = H // P              # 4 hidden chunks
NBLK = T // 512           # 8 column blocks of 512
HALF = K // 2             # 4
HT = T // 2               # 2048 columns per half-row
F32 = mybir.dt.float32
BF16 = mybir.dt.bfloat16
FP8 = mybir.dt.float8e4
I32 = mybir.dt.int32
BF = ml_dtypes.bfloat16
F8 = ml_dtypes.float8_e4m3
ALU = mybir.AluOpType
AF = mybir.ActivationFunctionType
PM = mybir.MatmulPerfMode

OSCALE = 8.0              # block output scaled by 8 before the fp8 write

_CACHE = {}
_REPEAT = 1    # timing-only knob: emit the whole pipeline N times in one NEFF
_PFX = [""]    # tile-name suffix per repeat (names must be unique)

# xbw layout: copy A (padded x, 4105 elems) at 0, copy B (=A shifted by
# one element) at XBOFF (16-aligned so the DoubleRow pair stride is legal)
XA = 4105
XBOFF = 4112
XW = XBOFF + 4104

# cpack layout
_NF32 = 2 + 4 + 4 + 2                          # dwb, b1f, gam8, b2c8
_BF0 = _NF32 * 2                               # bf16 elem offset (=24)
_NBF = 1024 + 1 + 128                          # w2t, ones_col, ones_row
_F80 = _NF32 * 4 + _NBF * 2                    # fp8 byte offset (=2354)
_DG = _F80                                     # diag pairs: 4 pairs x 2cc x 256
_D8 = _DG + 2048                               # tap-8 diags: 2cc x 128
_W1 = _D8 + 256                                # w1pk: 1024
_O16 = _W1 + 1024                              # ones16: 17
CPB = _O16 + 17
CPB += (-CPB) % 4


def _ap3(t, off, s1, n2, s2):
    """[128, 2, n2] AP over tile t at element offset off (pair stride s1,
    inner stride s2) -- the 3D form DoubleRow matmuls consume."""
    v = t[:, off:off + 1]
    c = v.copy()
    pstride = list(c.ap[0])
    c.ap[:] = [pstride, [s1, 2], [s2, n2]]
    return c


def _rsqrt(nc, pool, v, pdim, n, tag):
    """Newton rsqrt on DVE for a small [pdim, n] f32 tile (avoids the ACT
    sqrt table set; gelu set stays resident)."""
    vi = pool.tile([pdim, n], I32, tag=f"{tag}_i", name=f"{tag}_i")
    nc.vector.tensor_scalar(
        out=vi, in0=v.bitcast(I32), scalar1=1, scalar2=None,
        op0=ALU.logical_shift_right,
    )
    nc.vector.tensor_scalar(out=vi, in0=vi, scalar1=0x5F3759DF, scalar2=-1,
                            op0=ALU.subtract, op1=ALU.mult)
    r = pool.tile([pdim, n], F32, tag=f"{tag}_r", name=f"{tag}_r")
    nc.vector.tensor_copy(out=r, in_=vi.bitcast(F32))
    h = pool.tile([pdim, n], F32, tag=f"{tag}_h", name=f"{tag}_h")
    for _ in range(3):
        nc.vector.tensor_mul(out=h, in0=r, in1=r)
        nc.vector.tensor_mul(out=h, in0=h, in1=v)
        nc.vector.tensor_scalar(
            out=h, in0=h, scalar1=-0.5, scalar2=1.5, op0=ALU.mult, op1=ALU.add
        )
        nc.vector.tensor_mul(out=r, in0=r, in1=h)
    return r


def _build():
    nc = bacc.Bacc(
        "TRN2", target_bir_lowering=False, debug=False, num_devices=NCORES
    )
    x_d = nc.dram_tensor("x", [BL, C, T], FP8, kind="ExternalInput").ap()
    cpack_d = nc.dram_tensor("cpack", [P, CPB], mybir.dt.uint8,
                             kind="ExternalInput").ap()
    out_d = nc.dram_tensor("out", [BL, C, T], FP8, kind="ExternalOutput").ap()

    with tile.TileContext(nc) as tc:
        with ExitStack() as ctx:
            _emit(ctx, tc, nc, x_d, out_d, cpack_d)
    nc.compile()
    return nc


def _emit(ctx, tc, nc, x_d, out_d, cpack_d):
    const = ctx.enter_context(tc.tile_pool(name="const", bufs=1))
    xb_p = ctx.enter_context(tc.tile_pool(name="xb", bufs=4))
    y_p = ctx.enter_context(tc.tile_pool(name="y", bufs=2))
    ysq_p = ctx.enter_context(tc.tile_pool(name="ysq", bufs=1))
    ypk_p = ctx.enter_context(tc.tile_pool(name="ypk", bufs=2))
    tmp_p = ctx.enter_context(tc.tile_pool(name="tmp", bufs=2))
    hid_p = ctx.enter_context(tc.tile_pool(name="hid", bufs=2))
    sm_p = ctx.enter_context(tc.tile_pool(name="sm", bufs=2))
    row_p = ctx.enter_context(tc.tile_pool(name="row", bufs=1))
    w2s_p = ctx.enter_context(tc.tile_pool(name="w2s", bufs=2))
    ob_p = ctx.enter_context(tc.tile_pool(name="ob", bufs=3))

    dw_ps = ctx.enter_context(tc.tile_pool(name="dwps", bufs=2, space="PSUM"))
    st_ps = ctx.enter_context(tc.tile_pool(name="stps", bufs=1, space="PSUM"))
    mm_ps = ctx.enter_context(tc.tile_pool(name="mmps", bufs=3, space="PSUM"))
    rep_ps = ctx.enter_context(tc.tile_pool(name="repps", bufs=2, space="PSUM"))

    # ---- constants: ONE packed DMA, then bitcast slices ----
    cp = const.tile([P, CPB], mybir.dt.uint8)
    nc.sync.dma_start(out=cp, in_=cpack_d)
    cpf = cp.bitcast(F32)
    dwb_s = cpf[:, 0:2]
    b1f_s = cpf[:, 2:6]
    gam_s = cpf[:, 6:10]              # grn gamma, pre-scaled by OSCALE
    b2c_s = cpf[:, 10:12]             # pw2 bias (+W2@grn_beta), pre-scaled
    cpb = cp.bitcast(BF16)
    w2t_s = cpb[:, _BF0:_BF0 + 1024]
    ones_col = cpb[:, _BF0 + 1024:_BF0 + 1025]
    ones_row = cpb[0:1, _BF0 + 1025:_BF0 + 1025 + P]
    cp8 = cp.bitcast(FP8)

    xb = {}       # (s, cc) -> fp8 [P, XW] padded input (copies A and B)
    y8 = {}       # s -> bf16 [P, 2T]  (cc-major, raw dwconv out)
    ypks = {}     # s -> fp8 [P, 2T]  (normed, the pw1 DoubleRow operand)
    hid = {}      # s -> fp8 [P, 4T]  (hc-major)
    rows = {}     # s -> (r_row, nmr_row) bf16 [1, T]
    w2s = {}      # s -> scaled pw2 lhsT (fp8)

    def load(s):
        for cc in range(NCC):
            t = xb_p.tile([P, XW], FP8, tag="xb", name=f"xb_{s}_{cc}{_PFX[0]}")
            xb[(s, cc)] = t
            src = x_d[s, cc * P:(cc + 1) * P, :]
            nc.sync.dma_start(out=t[:, HALF:HALF + T], in_=src)
            nc.sync.dma_start(out=t[:, XBOFF + 3:XBOFF + 3 + T], in_=src)
            # halos: A = [x0 x0 x0 x0 | x | x_ x_ x_ x_ x_], B = A shifted 1
            nc.vector.tensor_copy(
                out=t[:, 0:HALF],
                in_=t[:, HALF:HALF + 1].to_broadcast((P, HALF)))
            nc.vector.tensor_copy(
                out=t[:, HALF + T:XA],
                in_=t[:, HALF + T - 1:HALF + T].to_broadcast((P, XA - HALF - T)))
            nc.vector.tensor_copy(
                out=t[:, XBOFF:XBOFF + 3],
                in_=t[:, XBOFF + 3:XBOFF + 4].to_broadcast((P, 3)))
            nc.vector.tensor_copy(
                out=t[:, XBOFF + 3 + T:XW],
                in_=t[:, XBOFF + 2 + T:XBOFF + 3 + T].to_broadcast(
                    (P, XW - XBOFF - 3 - T)))

    def ln_half(s, hf, sqt, r_row, nmr_row):
        # LN math for one T-half on compact [16,128] tiles; emitted as soon
        # as that half's stats are drained so the rep matmuls never stall.
        HL = T // 2
        s_c = sm_p.tile([16, P], BF16, tag=f"s_c{hf}", name=f"s_c_{s}_{hf}{_PFX[0]}")
        q_c = sm_p.tile([16, P], BF16, tag=f"q_c{hf}", name=f"q_c_{s}_{hf}{_PFX[0]}")
        nc.sync.dma_start(out=s_c, in_=sqt[0:1, hf * HL:(hf + 1) * HL])
        nc.sync.dma_start(out=q_c, in_=sqt[32:33, hf * HL:(hf + 1) * HL])
        mu = sm_p.tile([16, P], F32, tag=f"mu{hf}")
        nc.vector.tensor_scalar(out=mu, in0=s_c, scalar1=1.0 / C, scalar2=None,
                                op0=ALU.mult)
        var = sm_p.tile([16, P], F32, tag=f"var{hf}")
        nc.vector.tensor_mul(out=var, in0=mu, in1=mu)
        nc.vector.scalar_tensor_tensor(
            out=var, in0=q_c, scalar=1.0 / C, in1=var,
            op0=ALU.mult, op1=ALU.subtract)
        nc.vector.tensor_scalar(out=var, in0=var, scalar1=1e-5, scalar2=None,
                                op0=ALU.add)
        r = _rsqrt(nc, sm_p, var, 16, P, f"rs{hf}")
        nmr = sm_p.tile([16, P], F32, tag=f"nmr{hf}")
        nc.vector.scalar_tensor_tensor(out=nmr, in0=mu, scalar=-1.0, in1=r,
                                       op0=ALU.mult, op1=ALU.mult)
        r_bf = sm_p.tile([16, P], BF16, tag=f"r_bf{hf}")
        nc.vector.tensor_copy(out=r_bf, in_=r)
        nmr_bf = sm_p.tile([16, P], BF16, tag=f"nmr_bf{hf}")
        nc.vector.tensor_copy(out=nmr_bf, in_=nmr)
        nc.sync.dma_start(out=r_row[:, hf * HL:(hf + 1) * HL], in_=r_bf)
        nc.sync.dma_start(out=nmr_row[:, hf * HL:(hf + 1) * HL], in_=nmr_bf)

    def dw_stats(s):
        hid[s] = hid_p.tile([P, 4 * T], FP8, tag="hid", name=f"hid_{s}{_PFX[0]}")
        yt = y_p.tile([P, 2 * T], BF16, tag="y", name=f"y_{s}{_PFX[0]}")
        y8[s] = yt
        ypk = ypk_p.tile([P, 2 * T], FP8, tag="ypk", name=f"ypk_{s}{_PFX[0]}")
        ypks[s] = ypk
        ysq = ysq_p.tile([P, 2 * T], BF16, tag="ysq", name=f"ysq_{s}{_PFX[0]}")
        sqt = row_p.tile([33, T], BF16, tag="sqt", name=f"sqt_{s}{_PFX[0]}")
        r_row = row_p.tile([1, T], BF16, tag="r_row", name=f"r_row_{s}{_PFX[0]}")
        nmr_row = row_p.tile([1, T], BF16, tag="nmr_row",
                             name=f"nmr_row_{s}{_PFX[0]}")
        rows[s] = (r_row, nmr_row)
        for hh in range(2):
            for sb in range(NBLK // 2):
                blk = hh * (NBLK // 2) + sb
                lo = blk * 512
                for cc in range(NCC):
                    xt = xb[(s, cc)]
                    ps = dw_ps.tile([P, 512], F32, tag="dwps")
                    for p_ in range(4):
                        nc.tensor.matmul(
                            ps,
                            lhsT=_ap3(cp8, _DG + (p_ * NCC + cc) * 256, 128, 128, 1),
                            rhs=_ap3(xt, lo + 2 * p_, XBOFF, 512, 1),
                            start=(p_ == 0), stop=False, perf_mode=PM.DoubleRow)
                    nc.tensor.matmul(
                        ps, lhsT=cp8[:, _D8 + cc * P:_D8 + (cc + 1) * P],
                        rhs=xt[:, lo + 8:lo + 8 + 512],
                        start=False, stop=True)
                    # drain psum + dw bias -> y bf16; split across ACT and
                    # DVE (psum reads are column-rate-bound on both)
                    if cc == 0:
                        nc.scalar.activation(
                            out=yt[:, cc * T + lo:cc * T + lo + 512], in_=ps,
                            func=AF.Identity, bias=dwb_s[:, cc:cc + 1],
                            scale=1.0)
                    else:
                        nc.vector.tensor_scalar(
                            out=yt[:, cc * T + lo:cc * T + lo + 512], in0=ps,
                            scalar1=dwb_s[:, cc:cc + 1], scalar2=None,
                            op0=ALU.add)
                for cc in range(NCC):
                    nc.vector.tensor_mul(
                        out=ysq[:, cc * T + lo:cc * T + lo + 512],
                        in0=yt[:, cc * T + lo:cc * T + lo + 512],
                        in1=yt[:, cc * T + lo:cc * T + lo + 512])
                # S and Q chains in different PE column groups -> they run
                # concurrently (DoubleRow rejects 1-partition dst, so plain
                # fp8 matmuls per cc chunk; lhsT = ones16 col 0)
                st2 = st_ps.tile([64, 512], F32, tag="stps",
                                 name=f"st2_{s}_{blk}{_PFX[0]}")
                for cc in range(NCC):
                    nc.tensor.matmul(st2[0:1, :], lhsT=ones_col,
                                     rhs=yt[:, cc * T + lo:cc * T + lo + 512],
                                     start=(cc == 0), stop=(cc == NCC - 1),
                                     tile_position=(0, 0),
                                     skip_group_check=True)
                    nc.tensor.matmul(st2[32:33, :], lhsT=ones_col,
                                     rhs=ysq[:, cc * T + lo:cc * T + lo + 512],
                                     start=(cc == 0), stop=(cc == NCC - 1),
                                     tile_position=(0, 32),
                                     skip_group_check=True)
                # one drain covers S (row 0) and Q (row 32); psum-op cost
                # is column-rate-bound so 33 partitions ride along free
                nc.vector.tensor_copy(out=sqt[:, lo:lo + 512],
                                      in_=st2[0:33, :])
            ln_half(s, hh, sqt, r_row, nmr_row)
            norm_half(s, hh)
            pw1_half(s, hh)

    def norm_half(s, hh):
        # ypk = y*r + nmr (fp8, the DoubleRow pw1 operand); broadcasts via
        # PSUM-resident rep matmuls; emitted per T-half right after that
        # half's LN rows land so DVE fills the other half's stats window
        r_row, nmr_row = rows[s]
        yt = y8[s]
        ypk = ypks[s]
        for blk in range(hh * (NBLK // 2), (hh + 1) * (NBLK // 2)):
            lo = blk * 512
            r_ps = rep_ps.tile([P, 512], F32, tag="repps")
            nc.tensor.matmul(r_ps, lhsT=ones_row, rhs=r_row[:, lo:lo + 512],
                             start=True, stop=True)
            n_ps = rep_ps.tile([P, 512], F32, tag="repps")
            nc.tensor.matmul(n_ps, lhsT=ones_row, rhs=nmr_row[:, lo:lo + 512],
                             start=True, stop=True)
            for cc in range(NCC):
                ysl = yt[:, cc * T + lo:cc * T + lo + 512]
                tm = tmp_p.tile([P, 512], BF16, tag="tmp")
                nc.vector.tensor_mul(out=tm, in0=ysl, in1=r_ps)
                nc.vector.tensor_add(out=ypk[:, cc * T + lo:cc * T + lo + 512],
                                     in0=tm, in1=n_ps)

    gx2s = {}

    def pw1_half(s, hh):
        ht = hid[s]
        ypk = ypks[s]
        for hc in range(NHC):
            for blk in range(hh * (NBLK // 2), (hh + 1) * (NBLK // 2)):
                lo = blk * 512
                ps = mm_ps.tile([P, 512], F32, tag="mmps")
                nc.tensor.matmul(
                    ps, lhsT=_ap3(cp8, _W1 + hc * P, 512, 128, 1),
                    rhs=_ap3(ypk, lo, T, 512, 1),
                    start=True, stop=True, perf_mode=PM.DoubleRow)
                nc.scalar.activation(
                    out=ht[:, hc * T + lo:hc * T + lo + 512],
                    in_=ps, func=AF.Gelu, bias=b1f_s[:, hc:hc + 1], scale=1.0)
    def gx2_acc(s):
        # GRN square+accum on ACT; y8(s) is dead after pw1 -> scratch
        ht = hid[s]
        yt = y8[s]
        gx2 = sm_p.tile([P, NHC], F32, tag="gx2", name=f"gx2_{s}{_PFX[0]}")
        gx2s[s] = gx2
        for hc in range(NHC):
            if s == BL - 1 and hc >= 2:
                sq = yt[:, 0:T].bitcast(BF16)
                nc.vector.tensor_mul(out=sq, in0=ht[:, hc * T:(hc + 1) * T],
                                     in1=ht[:, hc * T:(hc + 1) * T])
                nc.vector.tensor_reduce(out=gx2[:, hc:hc + 1], in_=sq,
                                        axis=mybir.AxisListType.X, op=ALU.add)
            else:
                nc.scalar.activation(out=yt[:, 0:T],
                                     in_=ht[:, hc * T:(hc + 1) * T],
                                     func=AF.Square,
                                     accum_out=gx2[:, hc:hc + 1])

    def grn(s):
        gx2 = gx2s[s]
        gx2f = sm_p.tile([P, NHC], F32, tag="gx2f")
        nc.vector.tensor_scalar(out=gx2f, in0=gx2, scalar1=1e-30, scalar2=None,
                                op0=ALU.add)
        rg = _rsqrt(nc, sm_p, gx2f, P, NHC, "rg")
        gx = sm_p.tile([P, NHC], F32, tag="gx")
        nc.vector.tensor_mul(out=gx, in0=gx2f, in1=rg)      # gx = sqrt(gx2)
        gx_bf = sm_p.tile([P, NHC], BF16, tag="gx_bf")
        nc.vector.tensor_copy(out=gx_bf, in_=gx)
        # mean over all H=512 channels: ones-matmul -> [1,4] -> reduce
        gt_ps = st_ps.tile([1, NHC], F32, tag="stps", name=f"gt_{s}{_PFX[0]}")
        nc.tensor.matmul(gt_ps, lhsT=ones_col, rhs=gx_bf,
                         start=True, stop=True)
        g_row = sm_p.tile([1, NHC], F32, tag="g_row")
        nc.vector.tensor_copy(out=g_row, in_=gt_ps)
        tot = sm_p.tile([1, 1], F32, tag="tot")
        nc.vector.tensor_reduce(out=tot, in_=g_row, axis=mybir.AxisListType.X,
                                op=ALU.add)
        nc.vector.tensor_scalar(out=tot, in0=tot, scalar1=1.0 / H,
                                scalar2=1e-6, op0=ALU.mult, op1=ALU.add)
        rm_row = sm_p.tile([1, 1], F32, tag="rm_row")
        nc.vector.reciprocal(out=rm_row, in_=tot)
        rm_bf = sm_p.tile([1, 1], BF16, tag="rm_bf")
        nc.vector.tensor_copy(out=rm_bf, in_=rm_row)
        rm_ps = st_ps.tile([P, 1], F32, tag="stps", name=f"rm_{s}{_PFX[0]}")
        nc.tensor.matmul(rm_ps, lhsT=ones_row, rhs=rm_bf,
                         start=True, stop=True)
        rm = sm_p.tile([P, 1], F32, tag="rm")
        nc.vector.tensor_copy(out=rm, in_=rm_ps)
        # a = OSCALE * (gamma*nx + 1); gamma arrives pre-scaled by OSCALE
        a = sm_p.tile([P, NHC], F32, tag="a")
        nc.vector.tensor_scalar(out=a, in0=gx, scalar1=rm, scalar2=None,
                                op0=ALU.mult)
        nc.vector.scalar_tensor_tensor(out=a, in0=a, scalar=1.0, in1=gam_s,
                                       op0=ALU.bypass, op1=ALU.mult)
        nc.vector.tensor_scalar(out=a, in0=a, scalar1=OSCALE, scalar2=None,
                                op0=ALU.add)
        w2s[s] = w2s_p.tile([P, NHC * C], FP8, tag="w2s", name=f"w2s_{s}{_PFX[0]}")
        for hc in range(NHC):
            nc.vector.tensor_scalar(
                out=w2s[s][:, hc * C:(hc + 1) * C],
                in0=w2t_s[:, hc * C:(hc + 1) * C],
                scalar1=a[:, hc:hc + 1], scalar2=None, op0=ALU.mult)

    def pw2(s, last=False):
        ht = hid[s]
        for cc in range(NCC):
            for ob_i in range(2):          # two [P, 2048] fp8 out tiles per cc
                ob = ob_p.tile([P, HT], FP8, tag="ob")
                for sub in range(4):
                    lo = (ob_i * 4 + sub) * 512
                    ps = mm_ps.tile([P, 512], F32, tag="mmps")
                    for j in range(2):     # hc pairs (0,1) and (2,3)
                        nc.tensor.matmul(
                            ps,
                            lhsT=_ap3(w2s[s], (2 * j) * C + cc * P, C, 128, 1),
                            rhs=_ap3(ht, (2 * j) * T + lo, T, 512, 1),
                            start=(j == 0), stop=(j == 1),
                            perf_mode=PM.DoubleRow)
                    # drain: fp8 out = psum + bias2 (scaled by OSCALE); on the
                    # final sample nothing overlaps, so split ACT/DVE
                    if last and sub % 2 == 1:
                        nc.vector.tensor_scalar(
                            out=ob[:, sub * 512:(sub + 1) * 512], in0=ps,
                            scalar1=b2c_s[:, cc:cc + 1], scalar2=None,
                            op0=ALU.add)
                    else:
                        nc.scalar.activation(
                            out=ob[:, sub * 512:(sub + 1) * 512], in_=ps,
                            func=AF.Identity, bias=b2c_s[:, cc:cc + 1],
                            scale=1.0)
                nc.sync.dma_start(
                    out=out_d[s, cc * P:(cc + 1) * P,
                              ob_i * HT:(ob_i + 1) * HT],
                    in_=ob)

    # deferred-GRN pipeline: iter s runs dw+stats(s) / grn+pw2(s-1) /
    # norm+pw1(s); gelu+square ACT tails of pw1(s) overlap dw(s+1), giving
    # the GRN chain a full iteration of slack before pw2(s) needs w2s.
    for rp in range(_REPEAT):
        _PFX[0] = f"_rp{rp}" if _REPEAT > 1 else ""
        load(0)
        for s in range(BL):
            if s + 1 < BL:
                load(s + 1)
            dw_stats(s)
            if s >= 1:
                grn(s - 1)
                pw2(s - 1)
            gx2_acc(s)
        grn(BL - 1)
        pw2(BL - 1, last=True)


def _prep_inputs(inputs):
    x = np.asarray(inputs["x"], np.float32)
    dw_w = np.asarray(inputs["dw_w"], np.float32)      # (C,1,K)
    dw_b = np.asarray(inputs["dw_b"], np.float32)
    ln_w = np.asarray(inputs["ln_w"], np.float32)
    ln_b = np.asarray(inputs["ln_b"], np.float32)
    pw1_w = np.asarray(inputs["pw1_w"], np.float32)    # (H,C)
    pw1_b = np.asarray(inputs["pw1_b"], np.float32)
    gg = np.asarray(inputs["grn_gamma"], np.float32)
    gb = np.asarray(inputs["grn_beta"], np.float32)
    pw2_w = np.asarray(inputs["pw2_w"], np.float32)    # (C,H)
    pw2_b = np.asarray(inputs["pw2_b"], np.float32)

    dwb = dw_b.reshape(NCC, P).T.copy()
    b1f = (pw1_b + pw1_w @ ln_b).reshape(NHC, P).T.copy()
    gam = (gg * OSCALE).reshape(NHC, P).T.copy()
    b2c = ((pw2_b + pw2_w @ gb) * OSCALE).reshape(NCC, P).T.copy()

    w2t = np.zeros((P, NHC * C), BF)
    for hc in range(NHC):
        w2t[:, hc * C:(hc + 1) * C] = \
            pw2_w[:, hc * P:(hc + 1) * P].T.astype(BF)
    onescol = np.ones((P, 1), BF)
    onesrow_blk = np.zeros((P, P), BF)
    onesrow_blk[0, :] = 1.0

    # fp8 block: dwconv diag pairs, tap-8 diags, w1pk, ones16
    dgpk = np.zeros((P, 2048), F8)
    for p_ in range(4):
        for cc in range(NCC):
            base = (p_ * NCC + cc) * 256
            for j in range(2):
                k = 2 * p_ + j
                dg = np.zeros((P, P), np.float32)
                np.fill_diagonal(dg, dw_w[cc * P:(cc + 1) * P, 0, k])
                dgpk[:, base + j * P:base + (j + 1) * P] = dg.astype(F8)
    d8 = np.zeros((P, 2 * P), F8)
    for cc in range(NCC):
        dg = np.zeros((P, P), np.float32)
        np.fill_diagonal(dg, dw_w[cc * P:(cc + 1) * P, 0, 8])
        d8[:, cc * P:(cc + 1) * P] = dg.astype(F8)
    w1f = pw1_w * ln_w[None, :]                        # (H,C)
    w1pk = np.zeros((P, 1024), F8)
    for cc in range(NCC):
        for hc in range(NHC):
            w1pk[:, cc * 512 + hc * P:cc * 512 + (hc + 1) * P] = \
                w1f[hc * P:(hc + 1) * P, cc * P:(cc + 1) * P].T.astype(F8)
    ones16 = np.zeros((P, 17), F8)
    ones16[:, 0] = 1.0
    ones16[:, 16] = 1.0

    cpack = np.concatenate([
        dwb.view(np.uint8), b1f.view(np.uint8), gam.view(np.uint8),
        b2c.view(np.uint8),
        w2t.view(np.uint8), onescol.view(np.uint8), onesrow_blk.view(np.uint8),
        dgpk.view(np.uint8), d8.view(np.uint8), w1pk.view(np.uint8),
        ones16.view(np.uint8)], axis=1)
    pad = CPB - cpack.shape[1]
    if pad:
        cpack = np.concatenate([cpack, np.zeros((P, pad), np.uint8)], axis=1)
    assert cpack.shape == (P, CPB), cpack.shape
    x8 = x.astype(F8)
    common = {"cpack": np.ascontiguousarray(cpack)}
    in_maps = []
    for i in range(NCORES):
        m = dict(common)
        m["x"] = x8[i * BL:(i + 1) * BL]
        in_maps.append(m)
    return in_maps, x


def kernel(**inputs):
    if "nc" not in _CACHE:
        _CACHE["nc"] = _build()
    nc = _CACHE["nc"]
    in_maps, x = _prep_inputs(inputs)
    res = run_bass_kernel_spmd(nc, in_maps, core_ids=list(range(NCORES)),
                               **_CACHE.get("run_kwargs", {}))
    _CACHE["last_result"] = res
    y8 = np.concatenate([np.asarray(res.results[i]["out"])
                         for i in range(NCORES)], axis=0)
    return x + y8.astype(np.float32) * (1.0 / OSCALE)


# revision 21
# speedup vs baseline: 1.1131x; 1.0045x over previous
"""ConvNeXtV2 block (B=32, C=256, T=4096, K=9, H=512) on 8 trn2 cores.

Data-parallel over batch: 4 samples per core, no collectives.
Measured NEFF span ~353us/core (v2 baseline: 486us) with 4x less I/O.

Design (v11):
- fp8(e4m3) I/O: x is quantized to fp8 on the host; the kernel returns
  only the block output y*8 in fp8 and the f32 residual "+x" is added
  on the host, so quantization never touches the dominant term (the
  block output is ~12x smaller than x, and the LN after the dwconv
  washes systematic quantization error). I/O is 67MB/call vs 268MB.
- fp8 DoubleRow matmuls (2 fp8 contraction rows per PE cell):
  * dwconv: 4 DoubleRow diag-pair matmuls + 1 plain fp8 matmul per
    (cc, 512-block) -- 5 column streams instead of 9. A tap pair
    (k, k+1) needs rhs rows at element stride 1, which the ifmap AP
    rejects (stride 1 hard-hangs the PE), so x is DMA'd TWICE into one
    SBUF tile: copy A (padded x) at 0, copy B (x shifted by one) at a
    16-aligned offset; the pair stride is then 4112. The extra HBM
    read is free - DMA is far from roofline.
  * pw1: one DoubleRow matmul contracts all of C=256 per (hc, blk),
    reading the normed-y fp8 tile with a (cc0,cc1) pair stride of T.
  * pw2: two DoubleRow matmuls contract H=512 per (cc, blk).
- LN stats: S and Q ones-matmuls into one [64,512] psum (S@p0, Q@p32
  via tile_position) drained with a single [33,512] copy (psum reads
  are column-rate-bound, ~1.4ns/col, so partitions ride free); LN math
  on compact [16,128] tiles with a Newton rsqrt on DVE.
- norm computes ypk = y*r + nmr (fp8) against PSUM-broadcast rows; the
  old rank-1 nmr fold into pw1 is gone (DoubleRow already halved pw1).
- software pipeline interleaved at T/2 granularity: dwconv+stats,
  ln, norm and pw1 all emitted per half; grn+pw2 of sample s-1 run
  between samples; psum drains are split across ACT and DVE (both
  read psum at the same rate); last-sample drains/squares rebalance
  to DVE which idles in the tail.
- no gpsimd/SWDGE anywhere; all DMAs are HWDGE (sync).
Host pre-folds ln_w/ln_b into pw1, grn_beta and the fp8 x8 scale into
the pw2 bias/gamma; dwconv taps and pw1 weights are pre-quantized to
fp8 on the host. End-to-end relative error ~5.8e-3 (gate: 2e-2).
"""

from contextlib import ExitStack

import ml_dtypes
import numpy as np

import concourse.bass as bass
import concourse.mybir as mybir
import concourse.tile as tile
from concourse import bacc
from concourse.bass_utils import run_bass_kernel_spmd

B, C, T, K, H = 32, 256, 4096, 9, 512
NCORES = 8
BL = B // NCORES          # samples per core
P = 128
NCC = C // P              # 2 channel chunks
NHC = H // P              # 4 hidden chunks
NBLK = T // 512           # 8 column blocks of 512
HALF = K // 2             # 4
HT = T // 2               # 2048 columns per half-row
F32 = mybir.dt.float32
BF16 = mybir.dt.bfloat16
FP8 = mybir.dt.float8e4
I32 = mybir.dt.int32
BF = ml_dtypes.bfloat16
F8 = ml_dtypes.float8_e4m3
ALU = mybir.AluOpType
AF = mybir.ActivationFunctionType
PM = mybir.MatmulPerfMode

OSCALE = 8.0              # block output scaled by 8 before the fp8 write

_CACHE = {}
_REPEAT = 1    # timing-only knob: emit the whole pipeline N times in one NEFF
_PFX = [""]    # tile-name suffix per repeat (names must be unique)

# xbw layout: copy A (padded x, 4105 elems) at 0, copy B (=A shifted by
# one element) at XBOFF (16-aligned so the DoubleRow pair stride is legal)
XA = 4105
XBOFF = 4112
XW = XBOFF + 4104

# cpack layout
_NF32 = 2 + 4 + 4 + 2                          # dwb, b1f, gam8, b2c8
_BF0 = _NF32 * 2                               # bf16 elem offset (=24)
_NBF = 1024 + 1 + 128                          # w2t, ones_col, ones_row
_F80 = _NF32 * 4 + _NBF * 2                    # fp8 byte offset (=2354)
_DG = _F80                                     # diag pairs: 4 pairs x 2cc x 256
_D8 = _DG + 2048                               # tap-8 diags: 2cc x 128
_W1 = _D8 + 256                                # w1pk: 1024
_O16 = _W1 + 1024                              # ones16: 17
CPB = _O16 + 17
CPB += (-CPB) % 4


def _ap3(t, off, s1, n2, s2):
    """[128, 2, n2] AP over tile t at element offset off (pair stride s1,
    inner stride s2) -- the 3D form DoubleRow matmuls consume."""
    v = t[:, off:off + 1]
    c = v.copy()
    pstride = list(c.ap[0])
    c.ap[:] = [pstride, [s1, 2], [s2, n2]]
    return c


def _rsqrt(nc, pool, v, pdim, n, tag):
    """Newton rsqrt on DVE for a small [pdim, n] f32 tile (avoids the ACT
    sqrt table set; gelu set stays resident)."""
    vi = pool.tile([pdim, n], I32, tag=f"{tag}_i", name=f"{tag}_i")
    nc.vector.tensor_scalar(
        out=vi, in0=v.bitcast(I32), scalar1=1, scalar2=None,
        op0=ALU.logical_shift_right,
    )
    nc.vector.tensor_scalar(out=vi, in0=vi, scalar1=0x5F3759DF, scalar2=-1,
                            op0=ALU.subtract, op1=ALU.mult)
    r = pool.tile([pdim, n], F32, tag=f"{tag}_r", name=f"{tag}_r")
    nc.vector.tensor_copy(out=r, in_=vi.bitcast(F32))
    h = pool.tile([pdim, n], F32, tag=f"{tag}_h", name=f"{tag}_h")
    for _ in range(3):
        nc.vector.tensor_mul(out=h, in0=r, in1=r)
        nc.vector.tensor_mul(out=h, in0=h, in1=v)
        nc.vector.tensor_scalar(
            out=h, in0=h, scalar1=-0.5, scalar2=1.5, op0=ALU.mult, op1=ALU.add
        )
        nc.vector.tensor_mul(out=r, in0=r, in1=h)
    return r


def _build():
    nc = bacc.Bacc(
        "TRN2", target_bir_lowering=False, debug=False, num_devices=NCORES
    )
    x_d = nc.dram_tensor("x", [BL, C, T], FP8, kind="ExternalInput").ap()
    cpack_d = nc.dram_tensor("cpack", [P, CPB], mybir.dt.uint8,
                             kind="ExternalInput").ap()
    out_d = nc.dram_tensor("out", [BL, C, T], FP8, kind="ExternalOutput").ap()

    with tile.TileContext(nc) as tc:
        with ExitStack() as ctx:
            _emit(ctx, tc, nc, x_d, out_d, cpack_d)
    nc.compile()
    return nc


def _emit(ctx, tc, nc, x_d, out_d, cpack_d):
    const = ctx.enter_context(tc.tile_pool(name="const", bufs=1))
    xb_p = ctx.enter_context(tc.tile_pool(name="xb", bufs=4))
    y_p = ctx.enter_context(tc.tile_pool(name="y", bufs=2))
    ysq_p = ctx.enter_context(tc.tile_pool(name="ysq", bufs=1))
    ypk_p = ctx.enter_context(tc.tile_pool(name="ypk", bufs=2))
    tmp_p = ctx.enter_context(tc.tile_pool(name="tmp", bufs=2))
    hid_p = ctx.enter_context(tc.tile_pool(name="hid", bufs=2))
    sm_p = ctx.enter_context(tc.tile_pool(name="sm", bufs=2))
    row_p = ctx.enter_context(tc.tile_pool(name="row", bufs=1))
    w2s_p = ctx.enter_context(tc.tile_pool(name="w2s", bufs=2))
    ob_p = ctx.enter_context(tc.tile_pool(name="ob", bufs=3))

    dw_ps = ctx.enter_context(tc.tile_pool(name="dwps", bufs=2, space="PSUM"))
    st_ps = ctx.enter_context(tc.tile_pool(name="stps", bufs=1, space="PSUM"))
    mm_ps = ctx.enter_context(tc.tile_pool(name="mmps", bufs=3, space="PSUM"))
    rep_ps = ctx.enter_context(tc.tile_pool(name="repps", bufs=2, space="PSUM"))

    # ---- constants: ONE packed DMA, then bitcast slices ----
    cp = const.tile([P, CPB], mybir.dt.uint8)
    nc.sync.dma_start(out=cp, in_=cpack_d)
    cpf = cp.bitcast(F32)
    dwb_s = cpf[:, 0:2]
    b1f_s = cpf[:, 2:6]
    gam_s = cpf[:, 6:10]              # grn gamma, pre-scaled by OSCALE
    b2c_s = cpf[:, 10:12]             # pw2 bias (+W2@grn_beta), pre-scaled
    cpb = cp.bitcast(BF16)
    w2t_s = cpb[:, _BF0:_BF0 + 1024]
    ones_col = cpb[:, _BF0 + 1024:_BF0 + 1025]
    ones_row = cpb[0:1, _BF0 + 1025:_BF0 + 1025 + P]
    cp8 = cp.bitcast(FP8)

    xb = {}       # (s, cc) -> fp8 [P, XW] padded input (copies A and B)
    y8 = {}       # s -> bf16 [P, 2T]  (cc-major, raw dwconv out)
    ypks = {}     # s -> fp8 [P, 2T]  (normed, the pw1 DoubleRow operand)
    hid = {}      # s -> fp8 [P, 4T]  (hc-major)
    rows = {}     # s -> (r_row, nmr_row) bf16 [1, T]
    w2s = {}      # s -> scaled pw2 lhsT (fp8)

    def load(s):
        for cc in range(NCC):
            t = xb_p.tile([P, XW], FP8, tag="xb", name=f"xb_{s}_{cc}{_PFX[0]}")
            xb[(s, cc)] = t
            src = x_d[s, cc * P:(cc + 1) * P, :]
            nc.sync.dma_start(out=t[:, HALF:HALF + T], in_=src)
            nc.sync.dma_start(out=t[:, XBOFF + 3:XBOFF + 3 + T], in_=src)
            # halos: A = [x0 x0 x0 x0 | x | x_ x_ x_ x_ x_], B = A shifted 1
            nc.vector.tensor_copy(
                out=t[:, 0:HALF],
                in_=t[:, HALF:HALF + 1].to_broadcast((P, HALF)))
            nc.vector.tensor_copy(
                out=t[:, HALF + T:XA],
                in_=t[:, HALF + T - 1:HALF + T].to_broadcast((P, XA - HALF - T)))
            nc.vector.tensor_copy(
                out=t[:, XBOFF:XBOFF + 3],
                in_=t[:, XBOFF + 3:XBOFF + 4].to_broadcast((P, 3)))
            nc.vector.tensor_copy(
                out=t[:, XBOFF + 3 + T:XW],
                in_=t[:, XBOFF + 2 + T:XBOFF + 3 + T].to_broadcast(
                    (P, XW - XBOFF - 3 - T)))

    def ln_half(s, hf, sqt, r_row, nmr_row):
        # LN math for one T-half on compact [16,128] tiles; emitted as soon
        # as that half's stats are drained so the rep matmuls never stall.
        HL = T // 2
        s_c = sm_p.tile([16, P], BF16, tag=f"s_c{hf}", name=f"s_c_{s}_{hf}{_PFX[0]}")
        q_c = sm_p.tile([16, P], BF16, tag=f"q_c{hf}", name=f"q_c_{s}_{hf}{_PFX[0]}")
        nc.sync.dma_start(out=s_c, in_=sqt[0:1, hf * HL:(hf + 1) * HL])
        nc.sync.dma_start(out=q_c, in_=sqt[32:33, hf * HL:(hf + 1) * HL])
        mu = sm_p.tile([16, P], F32, tag=f"mu{hf}")
        nc.vector.tensor_scalar(out=mu, in0=s_c, scalar1=1.0 / C, scalar2=None,
                                op0=ALU.mult)
        var = sm_p.tile([16, P], F32, tag=f"var{hf}")
        nc.vector.tensor_mul(out=var, in0=mu, in1=mu)
        nc.vector.scalar_tensor_tensor(
            out=var, in0=q_c, scalar=1.0 / C, in1=var,
            op0=ALU.mult, op1=ALU.subtract)
        nc.vector.tensor_scalar(out=var, in0=var, scalar1=1e-5, scalar2=None,
                                op0=ALU.add)
        r = _rsqrt(nc, sm_p, var, 16, P, f"rs{hf}")
        nmr = sm_p.tile([16, P], F32, tag=f"nmr{hf}")
        nc.vector.scalar_tensor_tensor(out=nmr, in0=mu, scalar=-1.0, in1=r,
                                       op0=ALU.mult, op1=ALU.mult)
        r_bf = sm_p.tile([16, P], BF16, tag=f"r_bf{hf}")
        nc.vector.tensor_copy(out=r_bf, in_=r)
        nmr_bf = sm_p.tile([16, P], BF16, tag=f"nmr_bf{hf}")
        nc.vector.tensor_copy(out=nmr_bf, in_=nmr)
        nc.sync.dma_start(out=r_row[:, hf * HL:(hf + 1) * HL], in_=r_bf)
        nc.sync.dma_start(out=nmr_row[:, hf * HL:(hf + 1) * HL], in_=nmr_bf)

    def dw_stats(s):
        hid[s] = hid_p.tile([P, 4 * T], FP8, tag="hid", name=f"hid_{s}{_PFX[0]}")
        yt = y_p.tile([P, 2 * T], BF16, tag="y", name=f"y_{s}{_PFX[0]}")
        y8[s] = yt
        ypk = ypk_p.tile([P, 2 * T], FP8, tag="ypk", name=f"ypk_{s}{_PFX[0]}")
        ypks[s] = ypk
        ysq = ysq_p.tile([P, 2 * T], BF16, tag="ysq", name=f"ysq_{s}{_PFX[0]}")
        sqt = row_p.tile([33, T], BF16, tag="sqt", name=f"sqt_{s}{_PFX[0]}")
        r_row = row_p.tile([1, T], BF16, tag="r_row", name=f"r_row_{s}{_PFX[0]}")
        nmr_row = row_p.tile([1, T], BF16, tag="nmr_row",
                             name=f"nmr_row_{s}{_PFX[0]}")
        rows[s] = (r_row, nmr_row)
        for hh in range(2):
            for sb in range(NBLK // 2):
                blk = hh * (NBLK // 2) + sb
                lo = blk * 512
                for cc in range(NCC):
                    xt = xb[(s, cc)]
                    ps = dw_ps.tile([P, 512], F32, tag="dwps")
                    for p_ in range(4):
                        nc.tensor.matmul(
                            ps,
                            lhsT=_ap3(cp8, _DG + (p_ * NCC + cc) * 256, 128, 128, 1),
                            rhs=_ap3(xt, lo + 2 * p_, XBOFF, 512, 1),
                            start=(p_ == 0), stop=False, perf_mode=PM.DoubleRow)
                    nc.tensor.matmul(
                        ps, lhsT=cp8[:, _D8 + cc * P:_D8 + (cc + 1) * P],
                        rhs=xt[:, lo + 8:lo + 8 + 512],
                        start=False, stop=True)
                    # drain psum + dw bias -> y bf16; split across ACT and
                    # DVE (psum reads are column-rate-bound on both)
                    if cc == 0:
                        nc.scalar.activation(
                            out=yt[:, cc * T + lo:cc * T + lo + 512], in_=ps,
                            func=AF.Identity, bias=dwb_s[:, cc:cc + 1],
                            scale=1.0)
                    else:
                        nc.vector.tensor_scalar(
                            out=yt[:, cc * T + lo:cc * T + lo + 512], in0=ps,
                            scalar1=dwb_s[:, cc:cc + 1], scalar2=None,
                            op0=ALU.add)
                for cc in range(NCC):
                    nc.vector.tensor_mul(
                        out=ysq[:, cc * T + lo:cc * T + lo + 512],
                        in0=yt[:, cc * T + lo:cc * T + lo + 512],
                        in1=yt[:, cc * T + lo:cc * T + lo + 512])
                # S and Q chains in different PE column groups -> they run
                # concurrently (DoubleRow rejects 1-partition dst, so plain
                # fp8 matmuls per cc chunk; lhsT = ones16 col 0)
                st2 = st_ps.tile([64, 512], F32, tag="stps",
                                 name=f"st2_{s}_{blk}{_PFX[0]}")
                for cc in range(NCC):
                    nc.tensor.matmul(st2[0:1, :], lhsT=ones_col,
                                     rhs=yt[:, cc * T + lo:cc * T + lo + 512],
                                     start=(cc == 0), stop=(cc == NCC - 1),
                                     tile_position=(0, 0),
                                     skip_group_check=True)
                    nc.tensor.matmul(st2[32:33, :], lhsT=ones_col,
                                     rhs=ysq[:, cc * T + lo:cc * T + lo + 512],
                                     start=(cc == 0), stop=(cc == NCC - 1),
                                     tile_position=(0, 32),
                                     skip_group_check=True)
                # one drain covers S (row 0) and Q (row 32); psum-op cost
                # is column-rate-bound so 33 partitions ride along free
                nc.vector.tensor_copy(out=sqt[:, lo:lo + 512],
                                      in_=st2[0:33, :])
            ln_half(s, hh, sqt, r_row, nmr_row)
            norm_half(s, hh)
            pw1_half(s, hh)

    def norm_half(s, hh):
        # ypk = y*r + nmr (fp8, the DoubleRow pw1 operand); broadcasts via
        # PSUM-resident rep matmuls; emitted per T-half right after that
        # half's LN rows land so DVE fills the other half's stats window
        r_row, nmr_row = rows[s]
        yt = y8[s]
        ypk = ypks[s]
        for blk in range(hh * (NBLK // 2), (hh + 1) * (NBLK // 2)):
            lo = blk * 512
            r_ps = rep_ps.tile([P, 512], F32, tag="repps")
            nc.tensor.matmul(r_ps, lhsT=ones_row, rhs=r_row[:, lo:lo + 512],
                             start=True, stop=True)
            n_ps = rep_ps.tile([P, 512], F32, tag="repps")
            nc.tensor.matmul(n_ps, lhsT=ones_row, rhs=nmr_row[:, lo:lo + 512],
                             start=True, stop=True)
            for cc in range(NCC):
                ysl = yt[:, cc * T + lo:cc * T + lo + 512]
                tm = tmp_p.tile([P, 512], BF16, tag="tmp")
                nc.vector.tensor_mul(out=tm, in0=ysl, in1=r_ps)
                nc.vector.tensor_add(out=ypk[:, cc * T + lo:cc * T + lo + 512],
                                     in0=tm, in1=n_ps)

    gx2s = {}

    def pw1_half(s, hh):
        ht = hid[s]
        ypk = ypks[s]
        for hc in range(NHC):
            for blk in range(hh * (NBLK // 2), (hh + 1) * (NBLK // 2)):
                lo = blk * 512
                ps = mm_ps.tile([P, 512], F32, tag="mmps")
                nc.tensor.matmul(
                    ps, lhsT=_ap3(cp8, _W1 + hc * P, 512, 128, 1),
                    rhs=_ap3(ypk, lo, T, 512, 1),
                    start=True, stop=True, perf_mode=PM.DoubleRow)
                nc.scalar.activation(
                    out=ht[:, hc * T + lo:hc * T + lo + 512],
                    in_=ps, func=AF.Gelu, bias=b1f_s[:, hc:hc + 1], scale=1.0)
    def gx2_acc(s):
        # GRN square+accum on ACT; y8(s) is dead after pw1 -> scratch
        ht = hid[s]
        yt = y8[s]
        gx2 = sm_p.tile([P, NHC], F32, tag="gx2", name=f"gx2_{s}{_PFX[0]}")
        gx2s[s] = gx2
        for hc in range(NHC):
            if s == BL - 1 and hc >= 2:
                sq = yt[:, 0:T].bitcast(BF16)
                nc.vector.tensor_mul(out=sq, in0=ht[:, hc * T:(hc + 1) * T],
                                     in1=ht[:, hc * T:(hc + 1) * T])
                nc.vector.tensor_reduce(out=gx2[:, hc:hc + 1], in_=sq,
                                        axis=mybir.AxisListType.X, op=ALU.add)
            else:
                nc.scalar.activation(out=yt[:, 0:T],
                                     in_=ht[:, hc * T:(hc + 1) * T],
                                     func=AF.Square,
                                     accum_out=gx2[:, hc:hc + 1])

    def grn(s):
        gx2 = gx2s[s]
        gx2f = sm_p.tile([P, NHC], F32, tag="gx2f")
        nc.vector.tensor_scalar(out=gx2f, in0=gx2, scalar1=1e-30, scalar2=None,
                                op0=ALU.add)
        rg = _rsqrt(nc, sm_p, gx2f, P, NHC, "rg")
        gx = sm_p.tile([P, NHC], F32, tag="gx")
        nc.vector.tensor_mul(out=gx, in0=gx2f, in1=rg)      # gx = sqrt(gx2)
        gx_bf = sm_p.tile([P, NHC], BF16, tag="gx_bf")
        nc.vector.tensor_copy(out=gx_bf, in_=gx)
        # mean over all H=512 channels: ones-matmul -> [1,4] -> reduce
        gt_ps = st_ps.tile([1, NHC], F32, tag="stps", name=f"gt_{s}{_PFX[0]}")
        nc.tensor.matmul(gt_ps, lhsT=ones_col, rhs=gx_bf,
                         start=True, stop=True)
        g_row = sm_p.tile([1, NHC], F32, tag="g_row")
        nc.vector.tensor_copy(out=g_row, in_=gt_ps)
        tot = sm_p.tile([1, 1], F32, tag="tot")
        nc.vector.tensor_reduce(out=tot, in_=g_row, axis=mybir.AxisListType.X,
                                op=ALU.add)
        nc.vector.tensor_scalar(out=tot, in0=tot, scalar1=1.0 / H,
                                scalar2=1e-6, op0=ALU.mult, op1=ALU.add)
        rm_row = sm_p.tile([1, 1], F32, tag="rm_row")
        nc.vector.reciprocal(out=rm_row, in_=tot)
        rm_bf = sm_p.tile([1, 1], BF16, tag="rm_bf")
        nc.vector.tensor_copy(out=rm_bf, in_=rm_row)
        rm_ps = st_ps.tile([P, 1], F32, tag="stps", name=f"rm_{s}{_PFX[0]}")
        nc.tensor.matmul(rm_ps, lhsT=ones_row, rhs=rm_bf,
                         start=True, stop=True)
        rm = sm_p.tile([P, 1], F32, tag="rm")
        nc.vector.tensor_copy(out=rm, in_=rm_ps)
        # a = OSCALE * (gamma*nx + 1); gamma arrives pre-scaled by OSCALE
        a = sm_p.tile([P, NHC], F32, tag="a")
        nc.vector.tensor_scalar(out=a, in0=gx, scalar1=rm, scalar2=None,
                                op0=ALU.mult)
        nc.vector.scalar_tensor_tensor(out=a, in0=a, scalar=1.0, in1=gam_s,
                                       op0=ALU.bypass, op1=ALU.mult)
        nc.vector.tensor_scalar(out=a, in0=a, scalar1=OSCALE, scalar2=None,
                                op0=ALU.add)
        w2s[s] = w2s_p.tile([P, NHC * C], FP8, tag="w2s", name=f"w2s_{s}{_PFX[0]}")
        for hc in range(NHC):
            nc.vector.tensor_scalar(
                out=w2s[s][:, hc * C:(hc + 1) * C],
                in0=w2t_s[:, hc * C:(hc + 1) * C],
                scalar1=a[:, hc:hc + 1], scalar2=None, op0=ALU.mult)

    def pw2(s, last=False):
        ht = hid[s]
        for cc in range(NCC):
            for ob_i in range(2):          # two [P, 2048] fp8 out tiles per cc
                ob = ob_p.tile([P, HT], FP8, tag="ob")
                for sub in range(4):
                    lo = (ob_i * 4 + sub) * 512
                    ps = mm_ps.tile([P, 512], F32, tag="mmps")
                    for j in range(2):     # hc pairs (0,1) and (2,3)
                        nc.tensor.matmul(
                            ps,
                            lhsT=_ap3(w2s[s], (2 * j) * C + cc * P, C, 128, 1),
                            rhs=_ap3(ht, (2 * j) * T + lo, T, 512, 1),
                            start=(j == 0), stop=(j == 1),
                            perf_mode=PM.DoubleRow)
                    # drain: fp8 out = psum + bias2 (scaled by OSCALE); on the
                    # final sample nothing overlaps, so split ACT/DVE
                    if last and sub % 2 == 1:
                        nc.vector.tensor_scalar(
                            out=ob[:, sub * 512:(sub + 1) * 512], in0=ps,
                            scalar1=b2c_s[:, cc:cc + 1], scalar2=None,
                            op0=ALU.add)
                    else:
                        nc.scalar.activation(
                            out=ob[:, sub * 512:(sub + 1) * 512], in_=ps,
                            func=AF.Identity, bias=b2c_s[:, cc:cc + 1],
                            scale=1.0)
                nc.sync.dma_start(
                    out=out_d[s, cc * P:(cc + 1) * P,
                              ob_i * HT:(ob_i + 1) * HT],
                    in_=ob)

    # deferred-GRN pipeline: iter s runs dw+stats(s) / grn+pw2(s-1) /
    # norm+pw1(s); gelu+square ACT tails of pw1(s) overlap dw(s+1), giving
    # the GRN chain a full iteration of slack before pw2(s) needs w2s.
    for rp in range(_REPEAT):
        _PFX[0] = f"_rp{rp}" if _REPEAT > 1 else ""
        load(0)
        for s in range(BL):
            if s + 1 < BL:
                load(s + 1)
            dw_stats(s)
            if s >= 1:
                grn(s - 1)
                pw2(s - 1)
            gx2_acc(s)
        grn(BL - 1)
        pw2(BL - 1, last=True)


def _prep_inputs(inputs):
    x = np.asarray(inputs["x"], np.float32)
    dw_w = np.asarray(inputs["dw_w"], np.float32)      # (C,1,K)
    dw_b = np.asarray(inputs["dw_b"], np.float32)
    ln_w = np.asarray(inputs["ln_w"], np.float32)
    ln_b = np.asarray(inputs["ln_b"], np.float32)
    pw1_w = np.asarray(inputs["pw1_w"], np.float32)    # (H,C)
    pw1_b = np.asarray(inputs["pw1_b"], np.float32)
    gg = np.asarray(inputs["grn_gamma"], np.float32)
    gb = np.asarray(inputs["grn_beta"], np.float32)
    pw2_w = np.asarray(inputs["pw2_w"], np.float32)    # (C,H)
    pw2_b = np.asarray(inputs["pw2_b"], np.float32)

    dwb = dw_b.reshape(NCC, P).T.copy()
    b1f = (pw1_b + pw1_w @ ln_b).reshape(NHC, P).T.copy()
    gam = (gg * OSCALE).reshape(NHC, P).T.copy()
    b2c = ((pw2_b + pw2_w @ gb) * OSCALE).reshape(NCC, P).T.copy()

    w2t = np.zeros((P, NHC * C), BF)
    for hc in range(NHC):
        w2t[:, hc * C:(hc + 1) * C] = \
            pw2_w[:, hc * P:(hc + 1) * P].T.astype(BF)
    onescol = np.ones((P, 1), BF)
    onesrow_blk = np.zeros((P, P), BF)
    onesrow_blk[0, :] = 1.0

    # fp8 block: dwconv diag pairs, tap-8 diags, w1pk, ones16
    dgpk = np.zeros((P, 2048), F8)
    for p_ in range(4):
        for cc in range(NCC):
            base = (p_ * NCC + cc) * 256
            for j in range(2):
                k = 2 * p_ + j
                dg = np.zeros((P, P), np.float32)
                np.fill_diagonal(dg, dw_w[cc * P:(cc + 1) * P, 0, k])
                dgpk[:, base + j * P:base + (j + 1) * P] = dg.astype(F8)
    d8 = np.zeros((P, 2 * P), F8)
    for cc in range(NCC):
        dg = np.zeros((P, P), np.float32)
        np.fill_diagonal(dg, dw_w[cc * P:(cc + 1) * P, 0, 8])
        d8[:, cc * P:(cc + 1) * P] = dg.astype(F8)
    w1f = pw1_w * ln_w[None, :]                        # (H,C)
    w1pk = np.zeros((P, 1024), F8)
    for cc in range(NCC):
        for hc in range(NHC):
            w1pk[:, cc * 512 + hc * P:cc * 512 + (hc + 1) * P] = \
                w1f[hc * P:(hc + 1) * P, cc * P:(cc + 1) * P].T.astype(F8)
    ones16 = np.zeros((P, 17), F8)
    ones16[:, 0] = 1.0
    ones16[:, 16] = 1.0

    cpack = np.concatenate([
        dwb.view(np.uint8), b1f.view(np.uint8), gam.view(np.uint8),
        b2c.view(np.uint8),
        w2t.view(np.uint8), onescol.view(np.uint8), onesrow_blk.view(np.uint8),
        dgpk.view(np.uint8), d8.view(np.uint8), w1pk.view(np.uint8),
        ones16.view(np.uint8)], axis=1)
    pad = CPB - cpack.shape[1]
    if pad:
        cpack = np.concatenate([cpack, np.zeros((P, pad), np.uint8)], axis=1)
    assert cpack.shape == (P, CPB), cpack.shape
    x8 = x.astype(F8)
    common = {"cpack": np.ascontiguousarray(cpack)}
    in_maps = []
    for i in range(NCORES):
        m = dict(common)
        m["x"] = x8[i * BL:(i + 1) * BL]
        in_maps.append(m)
    return in_maps, x


def kernel(**inputs):
    if "nc" not in _CACHE:
        _CACHE["nc"] = _build()
    nc = _CACHE["nc"]
    in_maps, x = _prep_inputs(inputs)
    res = run_bass_kernel_spmd(nc, in_maps, core_ids=list(range(NCORES)),
                               **_CACHE.get("run_kwargs", {}))
    _CACHE["last_result"] = res
    y8 = np.concatenate([np.asarray(res.results[i]["out"])
                         for i in range(NCORES)], axis=0)
    return x + y8.astype(np.float32) * (1.0 / OSCALE)
